# revision 12
# baseline (speedup 1.0000x reference)
"""CARAFE kernel for Trainium2 (8 NeuronCores, batch-parallel), bf16 pipeline.

Reference computation per image:
  R = relu(conv1x1(x, w_compress, b_compress))          [48, 128, 128]
  E = conv3x3(R, w_encoder, b_encoder, pad=1)           [100, 128, 128]
  Y = softmax over k of E.reshape(4, 25, H, W)          (s, k, h, w)
  out[s,c,h,w] = sum_k Y[s,k,h,w] * xpad[c, h+dy, w+dx] (k=(dy,dx), 5x5, pad 2)
  pixel-shuffle: out_ref[s*16 + c//4, 2h + (c//2)%2, 2w + c%2] = out[s,c,h,w]

Mapping (single pass, SBUF-resident, bf16 data / fp32 PSUM):
  - conv1x1 + conv3x3: TensorE matmuls (channel-major), biases via ones rows.
    R kept zero-padded [49, 130*130] so conv taps are free-dim offsets.
  - exp on ScalarE (PSUM -> SBUF bf16), F^T via 128 PE transposes written
    (value-duplicated) into pixel-major fr2 [h, (s,k,w,2)].
  - softmax denominator: DVE strided reduce over k; reciprocal on DVE;
    normalization applied in-place on GpSimd per s-group (fr2 *= recipZ).
    The duplicated innermost pair keeps every apply operand 4B-aligned so
    the DVE auto-selects its 2x bf16 mode.
  - apply on DVE in pixel-major [h, (c_hi, w, c_lo)] channel-pair layout:
    per (s, w-half): 25 muls + 19 in-group adds (bf16, 2x) building 5
    dy-partials, merged into an fp32 accumulator (5 ops at 1x) to keep the
    25-term sum accurate; dy taps select one of 5 partition-shifted
    X copies (host-prepared HBM rows), dx taps are free-dim offsets.
  - pixel-shuffle falls out of the channel-pair layout: output DMA writes
    (w, c_lo) runs of 256 contiguous bf16 per (c4, row).
"""

import sys

import numpy as np

sys.path.insert(0, "/opt/trn_rl_repo")

import ml_dtypes

import concourse.bass as bass
import concourse.mybir as mybir
import concourse.tile as tile
from concourse import bacc
from concourse.masks import make_identity

F32 = mybir.dt.float32
BF16 = mybir.dt.bfloat16
BF_NP = ml_dtypes.bfloat16

H = 128
W = 128
C = 64
CH = 32  # channel pairs
M = 48  # compressed channels
S2 = 4  # scale_factor**2
K2 = 25  # k_up**2
SK = 100
HW = H * W
RP = 130  # padded R row pitch
WQ = 132  # padded w for the interleaved X^T buffer
XF = CH * WQ * 2  # 8448 free elements of each xtd tile
F2 = SK * W * 2  # 25600 free elements of fr2
N_CORES = 8

MULT = mybir.AluOpType.mult


def _ap(t, extra_off, dims):
    """Raw AP on a tile handle `t` with free-offset `extra_off` (elements)
    and explicit [step, count] dims (dims[0] is the partition dim)."""
    base = t[:]
    return bass.AP(tensor=base.tensor, offset=base.offset + extra_off, ap=dims)


class _Pool:
    """Manually scoped tile pool."""

    def __init__(self, tc, **kw):
        self._cm = tc.tile_pool(**kw)
        self.pool = self._cm.__enter__()
        self._n = 0

    def tile(self, *a, tag=None, **kw):
        self._n += 1
        t = tag or f"t{self._n}"
        return self.pool.tile(*a, tag=t, name=t, **kw)

    def close(self):
        self._cm.__exit__(None, None, None)


def build_program(debug=False):
    nc = bacc.Bacc("TRN2", target_bir_lowering=False, debug=False)

    x_aug = nc.dram_tensor("x_aug", [C + 1, HW], BF16, kind="ExternalInput")
    w1t = nc.dram_tensor("w1t", [C + 1, M], BF16, kind="ExternalInput")
    wet = nc.dram_tensor("wet", [M + 1, 9 * SK], BF16, kind="ExternalInput")
    xtq = nc.dram_tensor("xtq", [WQ, XF], BF16, kind="ExternalInput")
    onesr = nc.dram_tensor("onesr", [1, RP * RP], BF16, kind="ExternalInput")
    out = nc.dram_tensor("out", [C, 2 * H, 2 * W], BF16, kind="ExternalOutput")
    dbg = {}
    if debug:
        dbg["F"] = nc.dram_tensor("dbgF", [SK, HW], BF16, kind="ExternalOutput")
        dbg["FR2"] = nc.dram_tensor("dbgFR2", [128, F2], BF16, kind="ExternalOutput")
        dbg["Z"] = nc.dram_tensor("dbgZ", [128, S2 * W], F32, kind="ExternalOutput")

    with tile.TileContext(nc) as tc:
        cp = _Pool(tc, name="consts", bufs=1)
        w1t_sb = cp.tile([C + 1, M], BF16)
        nc.sync.dma_start(w1t_sb[:], w1t.ap())
        wet_sb = cp.tile([M + 1, 9 * SK], BF16)
        nc.sync.dma_start(wet_sb[:], wet.ap())
        ident = cp.tile([SK, SK], BF16)
        make_identity(nc, ident[:])

        # fr2 [h, (s, k, w, 2)] outlives the whole mask pipeline
        fr2p = _Pool(tc, name="fr2", bufs=1)
        fr2 = fr2p.tile([128, F2], BF16)

        fp_ = _Pool(tc, name="fsb", bufs=1)
        f_sb = fp_.tile([SK, HW], BF16)

        rp_ = _Pool(tc, name="R", bufs=1)
        R = rp_.tile([M + 1, RP * RP], BF16)
        nc.gpsimd.memset(R[:], 0.0)
        nc.sync.dma_start(
            _ap(R, M * RP * RP, [[RP * RP, 1], [1, RP * RP]]), onesr.ap()
        )

        pin = _Pool(tc, name="xin", bufs=1)
        x_sb = pin.tile([C + 1, HW], BF16)
        nc.sync.dma_start(x_sb[:], x_aug.ap())

        # ---- conv1x1 + relu -> R (strided interior writes) ----
        psA = _Pool(tc, name="psA", bufs=3, space="PSUM")
        for n in range(32):  # 4 image rows per chunk
            ps = psA.tile([M, 512], F32, tag="ps1")
            nc.tensor.matmul(
                ps[:], w1t_sb[:], x_sb[:, n * 512 : (n + 1) * 512],
                start=True, stop=True,
            )
            nc.scalar.activation(
                _ap(R, (1 + 4 * n) * RP + 1, [[RP * RP, M], [RP, 4], [1, W]]),
                ps[:],
                mybir.ActivationFunctionType.Relu,
            )
        psA.close()
        pin.close()

        # ---- conv3x3 (9 accumulating taps) + exp -> f_sb ----
        psB = _Pool(tc, name="psB", bufs=3, space="PSUM")
        for n in range(32):
            ps = psB.tile([SK, 512], F32, tag="ps2")
            for t in range(9):
                ty, tx = divmod(t, 3)
                nc.tensor.matmul(
                    ps[:],
                    wet_sb[:, t * SK : (t + 1) * SK],
                    _ap(R, (4 * n + ty) * RP + tx, [[RP * RP, M + 1], [RP, 4], [1, W]]),
                    start=(t == 0),
                    stop=(t == 8),
                )
            nc.scalar.activation(
                f_sb[:, n * 512 : (n + 1) * 512], ps[:],
                mybir.ActivationFunctionType.Exp,
            )
        psB.close()
        rp_.close()
        if debug:
            nc.sync.dma_start(dbg["F"].ap(), f_sb[:])

        # ---- F^T via PE transposes, duplicated write into fr2 ----
        zp = _Pool(tc, name="z", bufs=1)
        zbuf = zp.tile([128, S2 * W], F32)
        psF = _Pool(tc, name="psF", bufs=4, space="PSUM")
        for w in range(W):
            pst = psF.tile([128, SK], BF16, tag="pst")
            nc.tensor.transpose(pst[:], _ap(f_sb, w, [[HW, SK], [W, H]]), ident[:])
            nc.scalar.copy(
                _ap(fr2, w * 2, [[F2, 128], [W * 2, SK], [1, 2]]),
                _ap(pst, 0, [[SK, 128], [1, SK], [0, 2]]),
            )
        psF.close()

        # ---- softmax denominator + reciprocal (from unnormalized fr2) ----
        for s in range(S2):
            nc.vector.reduce_sum(
                zbuf[:, s * W : (s + 1) * W],
                _ap(fr2, s * K2 * W * 2, [[F2, 128], [2, W], [W * 2, K2]]),
                axis=mybir.AxisListType.X,
            )
        nc.vector.reciprocal(zbuf[:], zbuf[:])
        if debug:
            nc.sync.dma_start(dbg["Z"].ap(), zbuf[:])

        # ---- normalize in place per s-group on GpSimd ----
        for s in range(S2):
            nc.gpsimd.tensor_tensor(
                _ap(fr2, s * K2 * W * 2,
                    [[F2, 128], [W * 2, K2], [2, W], [1, 2]]),
                _ap(fr2, s * K2 * W * 2,
                    [[F2, 128], [W * 2, K2], [2, W], [1, 2]]),
                _ap(zbuf, s * W,
                    [[S2 * W, 128], [0, K2], [1, W], [0, 2]]),
                op=MULT,
            )
        zp.close()
        fp_.close()
        if debug:
            nc.sync.dma_start(dbg["FR2"].ap(), fr2[:])

        # ---- 5 partition-shifted (dy) copies of the interleaved X^T ----
        xp_ = _Pool(tc, name="xt", bufs=1)
        xtd = []
        for i in range(5):  # dy = i - 2
            td = xp_.tile([128, XF], BF16, tag=f"xtd{i}")
            nc.sync.dma_start(td[:], xtq.ap()[i : i + 128, :])
            xtd.append(td)

        # ---- apply: per (s, w-half): 25 muls + in-group adds (bf16 2x),
        #      dy-partials merged in fp32 ----
        WH = W // 2  # 64 output w per half
        FH = CH * WH * 2  # 4096 free elements per half
        pp_ = _Pool(tc, name="pbuf", bufs=3)
        tp_ = _Pool(tc, name="tbuf", bufs=1)
        a32p = _Pool(tc, name="acc32", bufs=1)
        abfp = _Pool(tc, name="accbf", bufs=1)
        dims_h = [[FH, 128], [WH * 2, CH], [2, WH], [1, 2]]
        for s in range(S2):
            accbf = abfp.tile([128, CH * W * 2], BF16, tag="accbf")
            for half in range(2):
                acc32 = a32p.tile([128, FH], F32, tag="acc32")
                pprev = None
                for dyi in range(5):
                    p = pp_.tile([128, FH], BF16, tag="p")
                    for dxi in range(5):
                        k = dyi * 5 + dxi
                        in0 = _ap(
                            xtd[dyi], (half * WH + dxi) * 2,
                            [[XF, 128], [WQ * 2, CH], [2, WH], [1, 2]],
                        )
                        in1 = _ap(
                            fr2, ((s * K2 + k) * W + half * WH) * 2,
                            [[F2, 128], [0, CH], [2, WH], [1, 2]],
                        )
                        if dxi == 0:
                            nc.vector.tensor_mul(_ap(p, 0, dims_h), in0, in1)
                        else:
                            t = tp_.tile([128, FH], BF16, tag="t")
                            nc.vector.tensor_mul(_ap(t, 0, dims_h), in0, in1)
                            nc.vector.tensor_add(p[:], p[:], t[:])
                    # dy-partials merge in fp32 on the (otherwise idle) Pool
                    # engine, in parallel with the DVE building the next group
                    if dyi == 1:
                        nc.gpsimd.tensor_add(acc32[:], pprev[:], p[:])
                    elif dyi in (2, 3):
                        nc.gpsimd.tensor_add(acc32[:], acc32[:], p[:])
                    elif dyi == 4:
                        nc.gpsimd.tensor_add(
                            _ap(accbf, half * WH * 2,
                                [[CH * W * 2, 128], [W * 2, CH], [2, WH], [1, 2]]),
                            acc32[:], p[:],
                        )
                    pprev = p
            # pixel-shuffle output: c = 2*c_hi + c_lo; c4 = c_hi//2,
            # c2 = c_hi%2, c1 = c_lo; rows 2h+c2, cols (2w+c1) contiguous
            for c2 in range(2):
                dst = bass.AP(
                    tensor=out,
                    offset=(s * 16) * (4 * HW) + c2 * (2 * W),
                    ap=[[2 * (2 * W), 128], [4 * HW, 16], [1, 2 * W]],
                )
                src = _ap(
                    accbf, c2 * (W * 2),
                    [[CH * W * 2, 128], [2 * (W * 2), 16], [1, 2 * W]],
                )
                nc.sync.dma_start(dst, src)
        abfp.close()
        a32p.close()
        tp_.close()
        pp_.close()
        xp_.close()
        fr2p.close()
        cp.close()
    nc.compile()
    return nc, dbg


def host_inputs(x_img, w_compress, b_compress, w_encoder, b_encoder):
    """Per-core input map for one image [C, H, W] (all bf16)."""
    w1t = np.concatenate(
        [w_compress[:, :, 0, 0].T, b_compress[None, :]], axis=0
    ).astype(BF_NP)
    wet = np.zeros((M + 1, 9, SK), np.float32)
    for ty in range(3):
        for tx in range(3):
            wet[:M, ty * 3 + tx, :] = w_encoder[:, :, ty, tx].T
    wet[M, 4, :] = b_encoder
    x_aug = np.ones((C + 1, HW), np.float32)
    x_aug[:C] = x_img.reshape(C, HW)
    # xtq[j, c_hi, wq, c_lo] = xpad[2*c_hi + c_lo, j-2, wq-2]
    xtq = np.zeros((WQ, CH, WQ, 2), np.float32)
    xtq[2:130, :, 2:130, :] = (
        x_img.reshape(CH, 2, H, W).transpose(2, 0, 3, 1)
    )
    return {
        "x_aug": x_aug.astype(BF_NP),
        "w1t": w1t,
        "wet": wet.reshape(M + 1, 9 * SK).astype(BF_NP),
        "xtq": xtq.reshape(WQ, XF).astype(BF_NP),
        "onesr": np.ones((1, RP * RP), BF_NP),
    }


_CACHE = {}


def kernel(x, w_compress, b_compress, w_encoder, b_encoder):
    x = np.asarray(x, np.float32)
    if "nc" not in _CACHE:
        _CACHE["nc"], _ = build_program(debug=False)
    nc = _CACHE["nc"]
    in_maps = [
        host_inputs(
            x[i],
            np.asarray(w_compress, np.float32),
            np.asarray(b_compress, np.float32),
            np.asarray(w_encoder, np.float32),
            np.asarray(b_encoder, np.float32),
        )
        for i in range(N_CORES)
    ]
    from concourse.bass_utils import run_bass_kernel_spmd

    res = run_bass_kernel_spmd(nc, in_maps, core_ids=list(range(N_CORES)))
    return np.stack(
        [res.results[i]["out"].astype(np.float32) for i in range(N_CORES)], axis=0
    )


# revision 15
# speedup vs baseline: 1.0657x; 1.0657x over previous
"""CARAFE kernel for Trainium2 (8 NeuronCores, batch-parallel), bf16 pipeline.

Reference computation per image:
  R = relu(conv1x1(x, w_compress, b_compress))          [48, 128, 128]
  E = conv3x3(R, w_encoder, b_encoder, pad=1)           [100, 128, 128]
  Y = softmax over k of E.reshape(4, 25, H, W)          (s, k, h, w)
  out[s,c,h,w] = sum_k Y[s,k,h,w] * xpad[c, h+dy, w+dx] (k=(dy,dx), 5x5, pad 2)
  pixel-shuffle: out_ref[s*16 + c//4, 2h + (c//2)%2, 2w + c%2] = out[s,c,h,w]

Mapping (single pass, SBUF-resident, bf16 data / fp32 PSUM):
  - mask branch runs w-major in pipelined 16-column superblocks:
    conv1x1 -> relu -> R (zero-padded [49, 130*130]), conv3x3 as 9
    accumulating matmuls (weights loaded once per superblock), exp on
    ScalarE into a small ring, then PE transposes write each column
    (value-duplicated) into pixel-major fr2 [h, (s,k,w,2)].  Running
    w-major means transposes chase the convs with a 1-superblock lag and
    the big X^T apply buffers stream from HBM concurrently.
  - softmax denominator: DVE strided reduce over k; reciprocal on DVE;
    normalization applied in-place on GpSimd per s-group (fr2 *= recipZ)
    while the DVE is otherwise idle.  The duplicated innermost pair keeps
    every apply operand 4B-aligned so the DVE runs its 2x bf16 mode.
  - apply on DVE in pixel-major [h, (c_hi, w, c_lo)] channel-pair layout:
    per (s, w-half): 25 muls + 20 in-group adds (bf16, 2x) build 5
    dy-partials; the partials are accumulated in fp32 PSUM by identity
    matmuls on the (idle) TensorE and extracted to bf16 by ScalarE, so
    the long 25-term sum never rounds through bf16.  dy taps select one
    of 5 partition-shifted X copies, dx taps are free-dim offsets.
  - pixel-shuffle falls out of the channel-pair layout: output DMA writes
    (w, c_lo) runs of 256 contiguous bf16 per (c4, row).
"""

import sys

import numpy as np

sys.path.insert(0, "/opt/trn_rl_repo")

import ml_dtypes

import concourse.bass as bass
import concourse.mybir as mybir
import concourse.tile as tile
from concourse import bacc
from concourse.masks import make_identity

F32 = mybir.dt.float32
BF16 = mybir.dt.bfloat16
BF_NP = ml_dtypes.bfloat16

H = 128
W = 128
C = 64
CH = 32  # channel pairs
M = 48  # compressed channels
S2 = 4  # scale_factor**2
K2 = 25  # k_up**2
SK = 100
HW = H * W
RP = 130  # padded R row pitch
WQ = 132  # padded w for the interleaved X^T buffer
XF = CH * WQ * 2  # 8448 free elements of each xtd tile
F2 = SK * W * 2  # 25600 free elements of fr2
NSB = 8  # superblocks (16 w-columns each)
SBW = W // NSB  # 16 w-columns per superblock
SBF = SBW * H  # 2048 pixels per superblock
N_CORES = 8

MULT = mybir.AluOpType.mult


def _ap(t, extra_off, dims):
    """Raw AP on a tile handle `t` with free-offset `extra_off` (elements)
    and explicit [step, count] dims (dims[0] is the partition dim)."""
    base = t[:]
    return bass.AP(tensor=base.tensor, offset=base.offset + extra_off, ap=dims)


class _Pool:
    """Manually scoped tile pool."""

    def __init__(self, tc, **kw):
        self._cm = tc.tile_pool(**kw)
        self.pool = self._cm.__enter__()
        self._n = 0

    def tile(self, *a, tag=None, **kw):
        self._n += 1
        t = tag or f"t{self._n}"
        return self.pool.tile(*a, tag=t, name=t, **kw)

    def close(self):
        self._cm.__exit__(None, None, None)


def build_program(debug=False):
    nc = bacc.Bacc("TRN2", target_bir_lowering=False, debug=False)

    x_wm = nc.dram_tensor("x_wm", [C + 1, HW], BF16, kind="ExternalInput")
    w1t = nc.dram_tensor("w1t", [C + 1, M], BF16, kind="ExternalInput")
    wet = nc.dram_tensor("wet", [M + 1, 9 * SK], BF16, kind="ExternalInput")
    xtq = nc.dram_tensor("xtq", [WQ, XF], BF16, kind="ExternalInput")
    onesr = nc.dram_tensor("onesr", [1, RP * RP], BF16, kind="ExternalInput")
    out = nc.dram_tensor("out", [C, 2 * H, 2 * W], BF16, kind="ExternalOutput")
    dbg = {}
    if debug:
        dbg["FR2"] = nc.dram_tensor("dbgFR2", [128, F2], BF16, kind="ExternalOutput")
        dbg["Z"] = nc.dram_tensor("dbgZ", [128, S2 * W], F32, kind="ExternalOutput")

    with tile.TileContext(nc) as tc:
        cp = _Pool(tc, name="consts", bufs=1)
        w1t_sb = cp.tile([C + 1, M], BF16)
        nc.sync.dma_start(w1t_sb[:], w1t.ap())
        wet_sb = cp.tile([M + 1, 9 * SK], BF16)
        nc.sync.dma_start(wet_sb[:], wet.ap())
        ident = cp.tile([SK, SK], BF16)
        make_identity(nc, ident[:])
        id128 = cp.tile([128, 128], BF16)
        make_identity(nc, id128[:])

        # fr2 [h, (s, k, w, 2)] and the 5 dy-shifted X^T copies live for
        # the whole kernel; xtd streams from HBM under the conv pipeline.
        fr2p = _Pool(tc, name="fr2", bufs=1)
        fr2 = fr2p.tile([128, F2], BF16)
        xp_ = _Pool(tc, name="xt", bufs=1)
        xtd = []
        for i in range(5):  # dy = i - 2
            td = xp_.tile([128, XF], BF16, tag=f"xtd{i}")
            nc.sync.dma_start(td[:], xtq.ap()[i : i + 128, :])
            xtd.append(td)
        zp = _Pool(tc, name="z", bufs=1)
        zbuf = zp.tile([128, S2 * W], F32)

        # ---- mask branch, pipelined w-major superblocks ----
        rp_ = _Pool(tc, name="R", bufs=1)
        R = rp_.tile([M + 1, RP * RP], BF16)
        nc.gpsimd.memset(R[:], 0.0)
        nc.sync.dma_start(
            _ap(R, M * RP * RP, [[RP * RP, 1], [1, RP * RP]]), onesr.ap()
        )
        xrp = _Pool(tc, name="xring", bufs=2)
        frg = _Pool(tc, name="fring", bufs=2)
        psA = _Pool(tc, name="psA", bufs=2, space="PSUM")
        psB = _Pool(tc, name="psB", bufs=1, space="PSUM")
        psF = _Pool(tc, name="psF", bufs=2, space="PSUM")

        def conv1x1_sb(sb):
            """conv1x1 + relu for superblock sb -> R columns."""
            xs = xrp.tile([C + 1, SBF], BF16, tag="xs")
            nc.sync.dma_start(xs[:], x_wm.ap()[:, sb * SBF : (sb + 1) * SBF])
            for j in range(4):  # 4 chunks of 4 w-cols
                ps = psA.tile([M, 512], F32, tag="ps1")
                nc.tensor.matmul(
                    ps[:], w1t_sb[:], xs[:, j * 512 : (j + 1) * 512],
                    start=True, stop=True,
                )
                w0 = sb * SBW + 4 * j
                nc.scalar.activation(
                    _ap(R, RP + 1 + w0, [[RP * RP, M], [1, 4], [RP, H]]),
                    ps[:],
                    mybir.ActivationFunctionType.Relu,
                )

        def conv3x3_sb(sb):
            """conv3x3 + exp + transposes for superblock sb -> fr2 columns."""
            fs = frg.tile([SK, SBF], BF16, tag="fs")
            # tap-outer over the superblock's 4 chunks (4 PSUM banks)
            pss = [psB.tile([SK, 512], F32, tag=f"ps2_{j}") for j in range(4)]
            for t in range(9):
                ty, tx = divmod(t, 3)
                for j in range(4):
                    nc.tensor.matmul(
                        pss[j][:],
                        wet_sb[:, t * SK : (t + 1) * SK],
                        _ap(R, ty * RP + tx + sb * SBW + 4 * j,
                            [[RP * RP, M + 1], [1, 4], [RP, H]]),
                        start=(t == 0),
                        stop=(t == 8),
                    )
            for j in range(4):
                nc.scalar.activation(
                    fs[:, j * 512 : (j + 1) * 512],
                    pss[j][:],
                    mybir.ActivationFunctionType.Exp,
                )
            for wi in range(SBW):
                pst = psF.tile([128, SK], BF16, tag="pst")
                nc.tensor.transpose(
                    pst[:], fs[:, wi * H : (wi + 1) * H], ident[:]
                )
                nc.scalar.copy(
                    _ap(fr2, (sb * SBW + wi) * 2, [[F2, 128], [W * 2, SK], [1, 2]]),
                    _ap(pst, 0, [[SK, 128], [1, SK], [0, 2]]),
                )

        conv1x1_sb(0)
        for sb in range(1, NSB):
            conv1x1_sb(sb)
            conv3x3_sb(sb - 1)
        conv3x3_sb(NSB - 1)
        psF.close()
        psB.close()
        psA.close()
        frg.close()
        xrp.close()
        rp_.close()

        # ---- softmax denominator + reciprocal (from unnormalized fr2) ----
        for s in range(S2):
            nc.vector.reduce_sum(
                zbuf[:, s * W : (s + 1) * W],
                _ap(fr2, s * K2 * W * 2, [[F2, 128], [2, W], [W * 2, K2]]),
                axis=mybir.AxisListType.X,
            )
        nc.vector.reciprocal(zbuf[:], zbuf[:])
        if debug:
            nc.sync.dma_start(dbg["Z"].ap(), zbuf[:])

        # ---- normalize in place per s-group on GpSimd ----
        for s in range(S2):
            nc.gpsimd.tensor_tensor(
                _ap(fr2, s * K2 * W * 2,
                    [[F2, 128], [W * 2, K2], [2, W], [1, 2]]),
                _ap(fr2, s * K2 * W * 2,
                    [[F2, 128], [W * 2, K2], [2, W], [1, 2]]),
                _ap(zbuf, s * W,
                    [[S2 * W, 128], [0, K2], [1, W], [0, 2]]),
                op=MULT,
            )
        zp.close()
        if debug:
            nc.sync.dma_start(dbg["FR2"].ap(), fr2[:])

        # ---- apply: per (s, w-half): 25 muls + in-group adds on DVE,
        #      dy-partials accumulated in fp32 PSUM by TensorE ----
        WH = W // 2  # 64 output w per half
        FH = CH * WH * 2  # 4096 free elements per half
        pp_ = _Pool(tc, name="pbuf", bufs=2)
        tp_ = _Pool(tc, name="tbuf", bufs=1)
        abfp = _Pool(tc, name="accbf", bufs=1)
        psc = _Pool(tc, name="psacc", bufs=1, space="PSUM")
        dims_h = [[FH, 128], [WH * 2, CH], [2, WH], [1, 2]]
        for s in range(S2):
            accbf = abfp.tile([128, CH * W * 2], BF16, tag="accbf")
            for half in range(2):
                psacc = psc.tile([128, FH], F32, tag="psacc")
                for dyi in range(5):
                    p = pp_.tile([128, FH], BF16, tag="p")
                    for dxi in range(5):
                        k = dyi * 5 + dxi
                        in0 = _ap(
                            xtd[dyi], (half * WH + dxi) * 2,
                            [[XF, 128], [WQ * 2, CH], [2, WH], [1, 2]],
                        )
                        in1 = _ap(
                            fr2, ((s * K2 + k) * W + half * WH) * 2,
                            [[F2, 128], [0, CH], [2, WH], [1, 2]],
                        )
                        if dxi == 0:
                            nc.vector.tensor_mul(_ap(p, 0, dims_h), in0, in1)
                        else:
                            t = tp_.tile([128, FH], BF16, tag="t")
                            nc.vector.tensor_mul(_ap(t, 0, dims_h), in0, in1)
                            nc.vector.tensor_add(p[:], p[:], t[:])
                    # accumulate the partial into fp32 PSUM on TensorE
                    for sl in range(8):
                        nc.tensor.matmul(
                            psacc[:, sl * 512 : (sl + 1) * 512],
                            id128[:],
                            p[:, sl * 512 : (sl + 1) * 512],
                            start=(dyi == 0),
                            stop=(dyi == 4),
                            skip_group_check=True,
                        )
                nc.scalar.copy(
                    _ap(accbf, half * WH * 2,
                        [[CH * W * 2, 128], [W * 2, CH], [2, WH], [1, 2]]),
                    psacc[:],
                )
            # pixel-shuffle output: c = 2*c_hi + c_lo; c4 = c_hi//2,
            # c2 = c_hi%2, c1 = c_lo; rows 2h+c2, cols (2w+c1) contiguous
            for c2 in range(2):
                dst = bass.AP(
                    tensor=out,
                    offset=(s * 16) * (4 * HW) + c2 * (2 * W),
                    ap=[[2 * (2 * W), 128], [4 * HW, 16], [1, 2 * W]],
                )
                src = _ap(
                    accbf, c2 * (W * 2),
                    [[CH * W * 2, 128], [2 * (W * 2), 16], [1, 2 * W]],
                )
                nc.sync.dma_start(dst, src)
        psc.close()
        abfp.close()
        tp_.close()
        pp_.close()
        xp_.close()
        fr2p.close()
        cp.close()
    nc.compile()
    return nc, dbg


def host_inputs(x_img, w_compress, b_compress, w_encoder, b_encoder):
    """Per-core input map for one image [C, H, W] (all bf16)."""
    w1t = np.concatenate(
        [w_compress[:, :, 0, 0].T, b_compress[None, :]], axis=0
    ).astype(BF_NP)
    wet = np.zeros((M + 1, 9, SK), np.float32)
    for ty in range(3):
        for tx in range(3):
            wet[:M, ty * 3 + tx, :] = w_encoder[:, :, ty, tx].T
    wet[M, 4, :] = b_encoder
    x_wm = np.ones((C + 1, HW), np.float32)
    x_wm[:C] = x_img.transpose(0, 2, 1).reshape(C, HW)  # w-major
    # xtq[j, c_hi, wq, c_lo] = xpad[2*c_hi + c_lo, j-2, wq-2]
    xtq = np.zeros((WQ, CH, WQ, 2), np.float32)
    xtq[2:130, :, 2:130, :] = (
        x_img.reshape(CH, 2, H, W).transpose(2, 0, 3, 1)
    )
    return {
        "x_wm": x_wm.astype(BF_NP),
        "w1t": w1t,
        "wet": wet.reshape(M + 1, 9 * SK).astype(BF_NP),
        "xtq": xtq.reshape(WQ, XF).astype(BF_NP),
        "onesr": np.ones((1, RP * RP), BF_NP),
    }


_CACHE = {}


def kernel(x, w_compress, b_compress, w_encoder, b_encoder):
    x = np.asarray(x, np.float32)
    if "nc" not in _CACHE:
        _CACHE["nc"], _ = build_program(debug=False)
    nc = _CACHE["nc"]
    in_maps = [
        host_inputs(
            x[i],
            np.asarray(w_compress, np.float32),
            np.asarray(b_compress, np.float32),
            np.asarray(w_encoder, np.float32),
            np.asarray(b_encoder, np.float32),
        )
        for i in range(N_CORES)
    ]
    from concourse.bass_utils import run_bass_kernel_spmd

    res = run_bass_kernel_spmd(nc, in_maps, core_ids=list(range(N_CORES)))
    return np.stack(
        [res.results[i]["out"].astype(np.float32) for i in range(N_CORES)], axis=0
    )


# revision 17
# speedup vs baseline: 1.1944x; 1.1208x over previous
"""CARAFE kernel for Trainium2 (8 NeuronCores, batch-parallel), bf16 pipeline.

Reference computation per image:
  R = relu(conv1x1(x, w_compress, b_compress))          [48, 128, 128]
  E = conv3x3(R, w_encoder, b_encoder, pad=1)           [100, 128, 128]
  Y = softmax over k of E.reshape(4, 25, H, W)          (s, k, h, w)
  out[s,c,h,w] = sum_k Y[s,k,h,w] * xpad[c, h+dy, w+dx] (k=(dy,dx), 5x5, pad 2)
  pixel-shuffle: out_ref[s*16 + c//4, 2h + (c//2)%2, 2w + c%2] = out[s,c,h,w]

Mapping (single pass, SBUF-resident, bf16 data / fp32 PSUM):
  - mask branch runs w-major in pipelined 16-column superblocks:
    conv1x1 -> relu -> R (zero-padded [49, 130*130]), conv3x3 as 9
    accumulating matmuls (weights loaded once per superblock), exp on
    ScalarE into a small ring, then PE transposes write each column
    (value-duplicated) into pixel-major fr2 [h, (s,k,w,2)].  Running
    w-major means transposes chase the convs with a 1-superblock lag and
    the big X^T apply buffers stream from HBM concurrently.
  - softmax denominator: DVE strided reduce over k; reciprocal on DVE;
    normalization applied in-place on GpSimd per s-group (fr2 *= recipZ)
    while the DVE is otherwise idle.  The duplicated innermost pair keeps
    every apply operand 4B-aligned so the DVE runs its 2x bf16 mode.
  - apply on DVE in pixel-major [h, (c_hi, w, c_lo)] channel-pair layout:
    per (s, w-half): 25 muls + 20 in-group adds (bf16, 2x) build 5
    dy-partials; the partials are accumulated in fp32 PSUM by identity
    matmuls on the (idle) TensorE and extracted to bf16 by ScalarE, so
    the long 25-term sum never rounds through bf16.  dy taps select one
    of 5 partition-shifted X copies, dx taps are free-dim offsets.
  - pixel-shuffle falls out of the channel-pair layout: output DMA writes
    (w, c_lo) runs of 256 contiguous bf16 per (c4, row).
"""

import sys

import numpy as np

sys.path.insert(0, "/opt/trn_rl_repo")

import ml_dtypes

import concourse.bass as bass
import concourse.mybir as mybir
import concourse.tile as tile
from concourse import bacc
from concourse.masks import make_identity

F32 = mybir.dt.float32
BF16 = mybir.dt.bfloat16
BF_NP = ml_dtypes.bfloat16

H = 128
W = 128
C = 64
CH = 32  # channel pairs
M = 48  # compressed channels
S2 = 4  # scale_factor**2
K2 = 25  # k_up**2
SK = 100
HW = H * W
RP = 130  # padded R row pitch
WQ = 132  # padded w for the interleaved X^T buffer
XF = CH * WQ * 2  # 8448 free elements of each xtd tile
F2 = SK * W * 2  # 25600 free elements of fr2
NSB = 8  # superblocks (16 w-columns each)
SBW = W // NSB  # 16 w-columns per superblock
SBF = SBW * H  # 2048 pixels per superblock
N_CORES = 8

MULT = mybir.AluOpType.mult


def _ap(t, extra_off, dims):
    """Raw AP on a tile handle `t` with free-offset `extra_off` (elements)
    and explicit [step, count] dims (dims[0] is the partition dim)."""
    base = t[:]
    return bass.AP(tensor=base.tensor, offset=base.offset + extra_off, ap=dims)


class _Pool:
    """Manually scoped tile pool."""

    def __init__(self, tc, **kw):
        self._cm = tc.tile_pool(**kw)
        self.pool = self._cm.__enter__()
        self._n = 0

    def tile(self, *a, tag=None, **kw):
        self._n += 1
        t = tag or f"t{self._n}"
        return self.pool.tile(*a, tag=t, name=t, **kw)

    def close(self):
        self._cm.__exit__(None, None, None)


def build_program(debug=False):
    nc = bacc.Bacc("TRN2", target_bir_lowering=False, debug=False)

    x_wm = nc.dram_tensor("x_wm", [C + 1, HW], BF16, kind="ExternalInput")
    w1t = nc.dram_tensor("w1t", [C + 1, M], BF16, kind="ExternalInput")
    wet = nc.dram_tensor("wet", [M + 1, 9 * SK], BF16, kind="ExternalInput")
    xtq = nc.dram_tensor("xtq", [WQ, XF], BF16, kind="ExternalInput")
    onesr = nc.dram_tensor("onesr", [1, RP * RP], BF16, kind="ExternalInput")
    out = nc.dram_tensor("out", [C, 2 * H, 2 * W], BF16, kind="ExternalOutput")
    dbg = {}
    if debug:
        dbg["FR2"] = nc.dram_tensor("dbgFR2", [128, F2], BF16, kind="ExternalOutput")
        dbg["Z"] = nc.dram_tensor("dbgZ", [128, S2 * W], F32, kind="ExternalOutput")

    with tile.TileContext(nc) as tc:
        cp = _Pool(tc, name="consts", bufs=1)
        w1t_sb = cp.tile([C + 1, M], BF16)
        nc.sync.dma_start(w1t_sb[:], w1t.ap())
        wet_sb = cp.tile([M + 1, 9 * SK], BF16)
        nc.sync.dma_start(wet_sb[:], wet.ap())
        ident = cp.tile([SK, SK], BF16)
        make_identity(nc, ident[:])
        id128 = cp.tile([128, 128], BF16)
        make_identity(nc, id128[:])

        # fr2 [h, (s, k, w, 2)] and the 5 dy-shifted X^T copies live for
        # the whole kernel; xtd streams from HBM under the conv pipeline.
        fr2p = _Pool(tc, name="fr2", bufs=1)
        fr2 = fr2p.tile([128, F2], BF16)
        xp_ = _Pool(tc, name="xt", bufs=1)
        xtd = []
        for i in range(5):  # dy = i - 2
            td = xp_.tile([128, XF], BF16, tag=f"xtd{i}")
            nc.sync.dma_start(td[:], xtq.ap()[i : i + 128, :])
            xtd.append(td)
        zp = _Pool(tc, name="z", bufs=1)
        zbuf = zp.tile([128, S2 * W], F32)

        # ---- mask branch, pipelined w-major superblocks ----
        rp_ = _Pool(tc, name="R", bufs=1)
        R = rp_.tile([M + 1, RP * RP], BF16)
        nc.gpsimd.memset(R[:], 0.0)
        nc.sync.dma_start(
            _ap(R, M * RP * RP, [[RP * RP, 1], [1, RP * RP]]), onesr.ap()
        )
        xrp = _Pool(tc, name="xring", bufs=2)
        frg = _Pool(tc, name="fring", bufs=2)
        psA = _Pool(tc, name="psA", bufs=2, space="PSUM")
        psB = _Pool(tc, name="psB", bufs=1, space="PSUM")
        psF = _Pool(tc, name="psF", bufs=2, space="PSUM")

        def conv1x1_sb(sb):
            """conv1x1 + relu for superblock sb -> R columns."""
            xs = xrp.tile([C + 1, SBF], BF16, tag="xs")
            nc.sync.dma_start(xs[:], x_wm.ap()[:, sb * SBF : (sb + 1) * SBF])
            for j in range(4):  # 4 chunks of 4 w-cols
                ps = psA.tile([M, 512], F32, tag="ps1")
                nc.tensor.matmul(
                    ps[:], w1t_sb[:], xs[:, j * 512 : (j + 1) * 512],
                    start=True, stop=True,
                )
                w0 = sb * SBW + 4 * j
                nc.scalar.activation(
                    _ap(R, (1 + w0) * RP + 1, [[RP * RP, M], [RP, 4], [1, H]]),
                    ps[:],
                    mybir.ActivationFunctionType.Relu,
                )

        def conv3x3_sb(sb):
            """conv3x3 + exp + transposes for superblock sb -> fr2 columns."""
            fs = frg.tile([SK, SBF], BF16, tag="fs")
            # tap-outer over the superblock's 4 chunks (4 PSUM banks)
            pss = [psB.tile([SK, 512], F32, tag=f"ps2_{j}") for j in range(4)]
            for t in range(9):
                ty, tx = divmod(t, 3)
                for j in range(4):
                    nc.tensor.matmul(
                        pss[j][:],
                        wet_sb[:, t * SK : (t + 1) * SK],
                        _ap(R, (sb * SBW + 4 * j + tx) * RP + ty,
                            [[RP * RP, M + 1], [RP, 4], [1, H]]),
                        start=(t == 0),
                        stop=(t == 8),
                    )
            for j in range(4):
                nc.scalar.activation(
                    fs[:, j * 512 : (j + 1) * 512],
                    pss[j][:],
                    mybir.ActivationFunctionType.Exp,
                )
            for wi in range(SBW):
                pst = psF.tile([128, SK], BF16, tag="pst")
                nc.tensor.transpose(
                    pst[:], fs[:, wi * H : (wi + 1) * H], ident[:]
                )
                nc.scalar.copy(
                    _ap(fr2, (sb * SBW + wi) * 2, [[F2, 128], [W * 2, SK], [1, 2]]),
                    _ap(pst, 0, [[SK, 128], [1, SK], [0, 2]]),
                )

        conv1x1_sb(0)
        for sb in range(1, NSB):
            conv1x1_sb(sb)
            conv3x3_sb(sb - 1)
        conv3x3_sb(NSB - 1)
        psF.close()
        psB.close()
        psA.close()
        frg.close()
        xrp.close()
        rp_.close()

        # ---- softmax denominator + reciprocal (from unnormalized fr2) ----
        for s in range(S2):
            nc.vector.reduce_sum(
                zbuf[:, s * W : (s + 1) * W],
                _ap(fr2, s * K2 * W * 2, [[F2, 128], [2, W], [W * 2, K2]]),
                axis=mybir.AxisListType.X,
            )
        nc.vector.reciprocal(zbuf[:], zbuf[:])
        if debug:
            nc.sync.dma_start(dbg["Z"].ap(), zbuf[:])

        # ---- normalize in place per s-group on GpSimd ----
        for s in range(S2):
            nc.gpsimd.tensor_tensor(
                _ap(fr2, s * K2 * W * 2,
                    [[F2, 128], [W * 2, K2], [2, W], [1, 2]]),
                _ap(fr2, s * K2 * W * 2,
                    [[F2, 128], [W * 2, K2], [2, W], [1, 2]]),
                _ap(zbuf, s * W,
                    [[S2 * W, 128], [0, K2], [1, W], [0, 2]]),
                op=MULT,
            )
        zp.close()
        if debug:
            nc.sync.dma_start(dbg["FR2"].ap(), fr2[:])

        # ---- apply: per (s, w-half): 25 muls + in-group adds on DVE,
        #      dy-partials accumulated in fp32 PSUM by TensorE ----
        WH = W // 2  # 64 output w per half
        FH = CH * WH * 2  # 4096 free elements per half
        pp_ = _Pool(tc, name="pbuf", bufs=2)
        tp_ = _Pool(tc, name="tbuf", bufs=1)
        abfp = _Pool(tc, name="accbf", bufs=1)
        psc = _Pool(tc, name="psacc", bufs=1, space="PSUM")
        dims_h = [[FH, 128], [WH * 2, CH], [2, WH], [1, 2]]
        for s in range(S2):
            accbf = abfp.tile([128, CH * W * 2], BF16, tag="accbf")
            for half in range(2):
                psacc = psc.tile([128, FH], F32, tag="psacc")
                for dyi in range(5):
                    p = pp_.tile([128, FH], BF16, tag="p")
                    for dxi in range(5):
                        k = dyi * 5 + dxi
                        in0 = _ap(
                            xtd[dyi], (half * WH + dxi) * 2,
                            [[XF, 128], [WQ * 2, CH], [2, WH], [1, 2]],
                        )
                        in1 = _ap(
                            fr2, ((s * K2 + k) * W + half * WH) * 2,
                            [[F2, 128], [0, CH], [2, WH], [1, 2]],
                        )
                        if dxi == 0:
                            nc.vector.tensor_mul(_ap(p, 0, dims_h), in0, in1)
                        else:
                            t = tp_.tile([128, FH], BF16, tag="t")
                            nc.vector.tensor_mul(_ap(t, 0, dims_h), in0, in1)
                            nc.vector.tensor_add(p[:], p[:], t[:])
                    # accumulate the partial into fp32 PSUM on TensorE
                    for sl in range(8):
                        nc.tensor.matmul(
                            psacc[:, sl * 512 : (sl + 1) * 512],
                            id128[:],
                            p[:, sl * 512 : (sl + 1) * 512],
                            start=(dyi == 0),
                            stop=(dyi == 4),
                            skip_group_check=True,
                        )
                nc.scalar.copy(
                    _ap(accbf, half * WH * 2,
                        [[CH * W * 2, 128], [W * 2, CH], [2, WH], [1, 2]]),
                    psacc[:],
                )
            # pixel-shuffle output: c = 2*c_hi + c_lo; c4 = c_hi//2,
            # c2 = c_hi%2, c1 = c_lo; rows 2h+c2, cols (2w+c1) contiguous
            for c2 in range(2):
                dst = bass.AP(
                    tensor=out,
                    offset=(s * 16) * (4 * HW) + c2 * (2 * W),
                    ap=[[2 * (2 * W), 128], [4 * HW, 16], [1, 2 * W]],
                )
                src = _ap(
                    accbf, c2 * (W * 2),
                    [[CH * W * 2, 128], [2 * (W * 2), 16], [1, 2 * W]],
                )
                nc.sync.dma_start(dst, src)
        psc.close()
        abfp.close()
        tp_.close()
        pp_.close()
        xp_.close()
        fr2p.close()
        cp.close()
    nc.compile()
    return nc, dbg


def host_inputs(x_img, w_compress, b_compress, w_encoder, b_encoder):
    """Per-core input map for one image [C, H, W] (all bf16)."""
    w1t = np.concatenate(
        [w_compress[:, :, 0, 0].T, b_compress[None, :]], axis=0
    ).astype(BF_NP)
    wet = np.zeros((M + 1, 9, SK), np.float32)
    for ty in range(3):
        for tx in range(3):
            wet[:M, ty * 3 + tx, :] = w_encoder[:, :, ty, tx].T
    wet[M, 4, :] = b_encoder
    x_wm = np.ones((C + 1, HW), np.float32)
    x_wm[:C] = x_img.transpose(0, 2, 1).reshape(C, HW)  # w-major
    # xtq[j, c_hi, wq, c_lo] = xpad[2*c_hi + c_lo, j-2, wq-2]
    xtq = np.zeros((WQ, CH, WQ, 2), np.float32)
    xtq[2:130, :, 2:130, :] = (
        x_img.reshape(CH, 2, H, W).transpose(2, 0, 3, 1)
    )
    return {
        "x_wm": x_wm.astype(BF_NP),
        "w1t": w1t,
        "wet": wet.reshape(M + 1, 9 * SK).astype(BF_NP),
        "xtq": xtq.reshape(WQ, XF).astype(BF_NP),
        "onesr": np.ones((1, RP * RP), BF_NP),
    }


_CACHE = {}


def kernel(x, w_compress, b_compress, w_encoder, b_encoder):
    x = np.asarray(x, np.float32)
    if "nc" not in _CACHE:
        _CACHE["nc"], _ = build_program(debug=False)
    nc = _CACHE["nc"]
    in_maps = [
        host_inputs(
            x[i],
            np.asarray(w_compress, np.float32),
            np.asarray(b_compress, np.float32),
            np.asarray(w_encoder, np.float32),
            np.asarray(b_encoder, np.float32),
        )
        for i in range(N_CORES)
    ]
    from concourse.bass_utils import run_bass_kernel_spmd

    res = run_bass_kernel_spmd(nc, in_maps, core_ids=list(range(N_CORES)))
    return np.stack(
        [res.results[i]["out"].astype(np.float32) for i in range(N_CORES)], axis=0
    )


# revision 18
# speedup vs baseline: 1.2639x; 1.0582x over previous
"""CARAFE kernel for Trainium2 (8 NeuronCores, batch-parallel), bf16 pipeline.

Reference computation per image:
  R = relu(conv1x1(x, w_compress, b_compress))          [48, 128, 128]
  E = conv3x3(R, w_encoder, b_encoder, pad=1)           [100, 128, 128]
  Y = softmax over k of E.reshape(4, 25, H, W)          (s, k, h, w)
  out[s,c,h,w] = sum_k Y[s,k,h,w] * xpad[c, h+dy, w+dx] (k=(dy,dx), 5x5, pad 2)
  pixel-shuffle: out_ref[s*16 + c//4, 2h + (c//2)%2, 2w + c%2] = out[s,c,h,w]

Mapping (single pass, SBUF-resident, bf16 data / fp32 PSUM):
  - mask branch runs w-major in pipelined 16-column superblocks:
    conv1x1 -> relu -> R (zero-padded, w-major [49, 130*130]), conv3x3 as
    9 accumulating matmuls, exp on ScalarE into a small ring, then PE
    transposes write each column (value-duplicated) into pixel-major
    fr2 [h, (s,k,w,2)] holding UNNORMALIZED exp values.
  - the apply is split by w-half: the half-0 units only need fr2 columns
    from superblocks 0-3, so (with precise range dependencies) the DVE
    starts the apply while TensorE is still convolving superblocks 4-7.
    X^T copies are staged per half so both halves' working sets fit SBUF
    alongside the conv buffers.
  - softmax denominator: DVE strided reduce over k per w-half +
    reciprocal; normalization is folded into the apply epilogue.
  - apply on DVE in pixel-major [h, (c_hi, w, c_lo)] channel-pair layout
    (the duplicated pair keeps every operand 4B-aligned -> DVE 2x mode):
    per (s, half): 25 muls + 20 in-group adds (bf16, 2x) build 5
    dy-partials; partials are accumulated in fp32 PSUM by identity
    matmuls on TensorE; the DVE epilogue multiplies PSUM by recipZ
    (normalization) and rounds once to bf16.
  - pixel-shuffle falls out of the channel-pair layout: output DMA writes
    (w, c_lo) runs of 256 contiguous bf16 per (c4, row).
"""

import sys

import numpy as np

sys.path.insert(0, "/opt/trn_rl_repo")

import ml_dtypes

import concourse.bass as bass
import concourse.mybir as mybir
import concourse.tile as tile
from concourse import bacc
from concourse.masks import make_identity

F32 = mybir.dt.float32
BF16 = mybir.dt.bfloat16
BF_NP = ml_dtypes.bfloat16

H = 128
W = 128
C = 64
CH = 32  # channel pairs
M = 48  # compressed channels
S2 = 4  # scale_factor**2
K2 = 25  # k_up**2
SK = 100
HW = H * W
RP = 130  # padded R row pitch (w-major: row = w, col = h)
WQ = 132  # padded w for the interleaved X^T buffer
XF = CH * WQ * 2  # full xtq row: 8448 elements
WHQ = 70  # w-columns per half-staged X^T tile (64 + 2*3 halo/alignment)
XFH = CH * WHQ * 2  # 4480 elements per half tile
F2 = SK * W * 2  # 25600 free elements of fr2
NSB = 8  # superblocks (16 w-columns each)
SBW = W // NSB  # 16 w-columns per superblock
SBF = SBW * H  # 2048 pixels per superblock
WH = W // 2  # 64 output w per half
FH = CH * WH * 2  # 4096 free elements per (s, half) unit
N_CORES = 8


def _ap(t, extra_off, dims):
    """Raw AP on a tile handle `t` with free-offset `extra_off` (elements)
    and explicit [step, count] dims (dims[0] is the partition dim)."""
    base = t[:]
    return bass.AP(tensor=base.tensor, offset=base.offset + extra_off, ap=dims)


class _Pool:
    """Manually scoped tile pool."""

    def __init__(self, tc, **kw):
        self._cm = tc.tile_pool(**kw)
        self.pool = self._cm.__enter__()
        self._n = 0

    def tile(self, *a, tag=None, **kw):
        self._n += 1
        t = tag or f"t{self._n}"
        return self.pool.tile(*a, tag=t, name=t, **kw)

    def close(self):
        self._cm.__exit__(None, None, None)


def build_program(debug=False):
    nc = bacc.Bacc("TRN2", target_bir_lowering=False, debug=False)

    x_wm = nc.dram_tensor("x_wm", [C + 1, HW], BF16, kind="ExternalInput")
    w1t = nc.dram_tensor("w1t", [C + 1, M], BF16, kind="ExternalInput")
    wet = nc.dram_tensor("wet", [M + 1, 9 * SK], BF16, kind="ExternalInput")
    xtq = nc.dram_tensor("xtq", [WQ, XF], BF16, kind="ExternalInput")
    onesr = nc.dram_tensor("onesr", [1, RP * RP], BF16, kind="ExternalInput")
    out = nc.dram_tensor("out", [C, 2 * H, 2 * W], BF16, kind="ExternalOutput")
    dbg = {}
    if debug:
        dbg["FR2"] = nc.dram_tensor("dbgFR2", [128, F2], BF16, kind="ExternalOutput")
        dbg["Z"] = nc.dram_tensor("dbgZ", [128, S2 * W], F32, kind="ExternalOutput")

    with tile.TileContext(nc) as tc:
        cp = _Pool(tc, name="consts", bufs=1)
        w1t_sb = cp.tile([C + 1, M], BF16)
        nc.sync.dma_start(w1t_sb[:], w1t.ap())
        wet_sb = cp.tile([M + 1, 9 * SK], BF16)
        nc.sync.dma_start(wet_sb[:], wet.ap())
        ident = cp.tile([SK, SK], BF16)
        make_identity(nc, ident[:])
        id128 = cp.tile([128, 128], BF16)
        make_identity(nc, id128[:])

        fr2p = _Pool(tc, name="fr2", bufs=1)
        fr2 = fr2p.tile([128, F2], BF16)
        zp = _Pool(tc, name="z", bufs=1)
        zbuf = zp.tile([128, S2 * W], F32)

        def load_xtd(pool, half):
            """5 dy-shifted, half-w X^T tiles [128, CH*70*2]."""
            tiles = []
            for i in range(5):  # dy = i - 2
                td = pool.tile([128, XFH], BF16, tag=f"xtd{half}_{i}")
                nc.sync.dma_start(
                    td[:],
                    bass.AP(
                        tensor=xtq,
                        offset=i * XF + half * (WH - 2) * 2,
                        ap=[[XF, 128], [WQ * 2, CH], [1, WHQ * 2]],
                    ),
                )
                tiles.append(td)
            return tiles

        xh0p = _Pool(tc, name="xth0", bufs=1)
        xtd0 = load_xtd(xh0p, 0)

        # ---- mask branch, pipelined w-major superblocks ----
        rp_ = _Pool(tc, name="R", bufs=1)
        R = rp_.tile([M + 1, RP * RP], BF16)
        nc.gpsimd.memset(R[:], 0.0)
        nc.sync.dma_start(
            _ap(R, M * RP * RP, [[RP * RP, 1], [1, RP * RP]]), onesr.ap()
        )
        xrp = _Pool(tc, name="xring", bufs=2)
        frg = _Pool(tc, name="fring", bufs=2)
        psA = _Pool(tc, name="psA", bufs=2, space="PSUM")
        psB = _Pool(tc, name="psB", bufs=1, space="PSUM")
        psF = _Pool(tc, name="psF", bufs=2, space="PSUM")

        def conv1x1_sb(sb):
            """conv1x1 + relu for superblock sb -> R columns (w-major)."""
            xs = xrp.tile([C + 1, SBF], BF16, tag="xs")
            nc.sync.dma_start(xs[:], x_wm.ap()[:, sb * SBF : (sb + 1) * SBF])
            for j in range(4):  # 4 chunks of 4 w-cols
                ps = psA.tile([M, 512], F32, tag="ps1")
                nc.tensor.matmul(
                    ps[:], w1t_sb[:], xs[:, j * 512 : (j + 1) * 512],
                    start=True, stop=True,
                )
                w0 = sb * SBW + 4 * j
                nc.scalar.activation(
                    _ap(R, (1 + w0) * RP + 1, [[RP * RP, M], [RP, 4], [1, H]]),
                    ps[:],
                    mybir.ActivationFunctionType.Relu,
                )

        def conv3x3_sb(sb):
            """conv3x3 + exp + transposes for superblock sb -> fr2 columns."""
            fs = frg.tile([SK, SBF], BF16, tag="fs")
            # tap-outer over the superblock's 4 chunks (4 PSUM banks)
            pss = [psB.tile([SK, 512], F32, tag=f"ps2_{j}") for j in range(4)]
            for t in range(9):
                ty, tx = divmod(t, 3)
                for j in range(4):
                    nc.tensor.matmul(
                        pss[j][:],
                        wet_sb[:, t * SK : (t + 1) * SK],
                        _ap(R, (sb * SBW + 4 * j + tx) * RP + ty,
                            [[RP * RP, M + 1], [RP, 4], [1, H]]),
                        start=(t == 0),
                        stop=(t == 8),
                    )
            for j in range(4):
                nc.scalar.activation(
                    fs[:, j * 512 : (j + 1) * 512],
                    pss[j][:],
                    mybir.ActivationFunctionType.Exp,
                )
            for wi in range(SBW):
                pst = psF.tile([128, SK], BF16, tag="pst")
                nc.tensor.transpose(
                    pst[:], fs[:, wi * H : (wi + 1) * H], ident[:]
                )
                nc.scalar.copy(
                    _ap(fr2, (sb * SBW + wi) * 2, [[F2, 128], [W * 2, SK], [1, 2]]),
                    _ap(pst, 0, [[SK, 128], [1, SK], [0, 2]]),
                )

        conv1x1_sb(0)
        for sb in range(1, NSB):
            conv1x1_sb(sb)
            conv3x3_sb(sb - 1)
        conv3x3_sb(NSB - 1)
        psF.close()
        psB.close()
        psA.close()

        def z_half(half):
            """softmax denominator + reciprocal for w-columns of `half`."""
            for s in range(S2):
                nc.vector.reduce_sum(
                    zbuf[:, s * W + half * WH : s * W + (half + 1) * WH],
                    _ap(fr2, (s * K2 * W + half * WH) * 2,
                        [[F2, 128], [2, WH], [W * 2, K2]]),
                    axis=mybir.AxisListType.X,
                )
            nc.vector.reciprocal(
                _ap(zbuf, half * WH, [[S2 * W, 128], [W, S2], [1, WH]]),
                _ap(zbuf, half * WH, [[S2 * W, 128], [W, S2], [1, WH]]),
            )

        def apply_units(half, xtd, pp_, tp_, abfp, psc):
            """4 (s, half) apply units: DVE taps, PE partial-merges,
            normalizing DVE epilogue, pixel-shuffle DMA out."""
            dims_u = [[FH, 128], [WH * 2, CH], [2, WH], [1, 2]]
            base_dx = 0 if half == 0 else 2
            for s in range(S2):
                psacc = psc.tile([128, FH], F32, tag="psacc")
                for dyi in range(5):
                    p = pp_.tile([128, FH], BF16, tag="p")
                    for dxi in range(5):
                        k = dyi * 5 + dxi
                        in0 = _ap(
                            xtd[dyi], (base_dx + dxi) * 2,
                            [[XFH, 128], [WHQ * 2, CH], [2, WH], [1, 2]],
                        )
                        in1 = _ap(
                            fr2, ((s * K2 + k) * W + half * WH) * 2,
                            [[F2, 128], [0, CH], [2, WH], [1, 2]],
                        )
                        if dxi == 0:
                            nc.vector.tensor_mul(_ap(p, 0, dims_u), in0, in1)
                        else:
                            t = tp_.tile([128, FH], BF16, tag="t")
                            nc.vector.tensor_mul(_ap(t, 0, dims_u), in0, in1)
                            nc.vector.tensor_add(p[:], p[:], t[:])
                    # accumulate the partial into fp32 PSUM on TensorE
                    for sl in range(8):
                        nc.tensor.matmul(
                            psacc[:, sl * 512 : (sl + 1) * 512],
                            id128[:],
                            p[:, sl * 512 : (sl + 1) * 512],
                            start=(dyi == 0),
                            stop=(dyi == 4),
                            skip_group_check=True,
                        )
                # normalize + round once: accbf = psacc * recipZ
                accbf = abfp.tile([128, FH], BF16, tag="accbf")
                nc.vector.tensor_mul(
                    _ap(accbf, 0, dims_u),
                    _ap(psacc, 0, dims_u),
                    _ap(zbuf, s * W + half * WH,
                        [[S2 * W, 128], [0, CH], [1, WH], [0, 2]]),
                )
                # pixel-shuffle output: c = 2*c_hi + c_lo; c4 = c_hi//2,
                # c2 = c_hi%2, c1 = c_lo; rows 2h+c2, cols (2w+c1)
                for c2 in range(2):
                    dst = bass.AP(
                        tensor=out,
                        offset=(s * 16) * (4 * HW) + c2 * (2 * W) + half * (2 * WH),
                        ap=[[2 * (2 * W), 128], [4 * HW, 16], [1, 2 * WH]],
                    )
                    src = _ap(
                        accbf, c2 * (WH * 2),
                        [[FH, 128], [2 * (WH * 2), 16], [1, 2 * WH]],
                    )
                    nc.sync.dma_start(dst, src)

        # half 0 apply overlaps the tail of the conv pipeline
        pp_ = _Pool(tc, name="pbuf", bufs=3)
        tp_ = _Pool(tc, name="tbuf", bufs=1)
        abfp = _Pool(tc, name="accbf", bufs=2)
        psc = _Pool(tc, name="psacc", bufs=1, space="PSUM")
        z_half(0)
        apply_units(0, xtd0, pp_, tp_, abfp, psc)
        psc.close()
        abfp.close()
        tp_.close()
        pp_.close()
        frg.close()
        xrp.close()
        rp_.close()
        xh0p.close()

        # half 1: X^T half tiles stream in while half 0 finishes
        xh1p = _Pool(tc, name="xth1", bufs=1)
        xtd1 = load_xtd(xh1p, 1)
        pp2 = _Pool(tc, name="pbuf2", bufs=3)
        tp2 = _Pool(tc, name="tbuf2", bufs=1)
        abf2 = _Pool(tc, name="accbf2", bufs=2)
        psc2 = _Pool(tc, name="psacc2", bufs=1, space="PSUM")
        z_half(1)
        apply_units(1, xtd1, pp2, tp2, abf2, psc2)
        psc2.close()
        abf2.close()
        tp2.close()
        pp2.close()
        xh1p.close()

        if debug:
            nc.sync.dma_start(dbg["FR2"].ap(), fr2[:])
            nc.sync.dma_start(dbg["Z"].ap(), zbuf[:])
        zp.close()
        fr2p.close()
        cp.close()
    nc.compile()
    return nc, dbg


def host_inputs(x_img, w_compress, b_compress, w_encoder, b_encoder):
    """Per-core input map for one image [C, H, W] (all bf16)."""
    w1t = np.concatenate(
        [w_compress[:, :, 0, 0].T, b_compress[None, :]], axis=0
    ).astype(BF_NP)
    wet = np.zeros((M + 1, 9, SK), np.float32)
    for ty in range(3):
        for tx in range(3):
            wet[:M, ty * 3 + tx, :] = w_encoder[:, :, ty, tx].T
    wet[M, 4, :] = b_encoder
    x_wm = np.ones((C + 1, HW), np.float32)
    x_wm[:C] = x_img.transpose(0, 2, 1).reshape(C, HW)  # w-major
    # xtq[j, c_hi, wq, c_lo] = xpad[2*c_hi + c_lo, j-2, wq-2]
    xtq = np.zeros((WQ, CH, WQ, 2), np.float32)
    xtq[2:130, :, 2:130, :] = (
        x_img.reshape(CH, 2, H, W).transpose(2, 0, 3, 1)
    )
    return {
        "x_wm": x_wm.astype(BF_NP),
        "w1t": w1t,
        "wet": wet.reshape(M + 1, 9 * SK).astype(BF_NP),
        "xtq": xtq.reshape(WQ, XF).astype(BF_NP),
        "onesr": np.ones((1, RP * RP), BF_NP),
    }


_CACHE = {}


def kernel(x, w_compress, b_compress, w_encoder, b_encoder):
    x = np.asarray(x, np.float32)
    if "nc" not in _CACHE:
        _CACHE["nc"], _ = build_program(debug=False)
    nc = _CACHE["nc"]
    in_maps = [
        host_inputs(
            x[i],
            np.asarray(w_compress, np.float32),
            np.asarray(b_compress, np.float32),
            np.asarray(w_encoder, np.float32),
            np.asarray(b_encoder, np.float32),
        )
        for i in range(N_CORES)
    ]
    from concourse.bass_utils import run_bass_kernel_spmd

    res = run_bass_kernel_spmd(nc, in_maps, core_ids=list(range(N_CORES)))
    return np.stack(
        [res.results[i]["out"].astype(np.float32) for i in range(N_CORES)], axis=0
    )


# revision 22
# speedup vs baseline: 1.4434x; 1.1420x over previous
"""CARAFE kernel for Trainium2 (8 NeuronCores, batch-parallel), bf16 pipeline.

Reference computation per image:
  R = relu(conv1x1(x, w_compress, b_compress))          [48, 128, 128]
  E = conv3x3(R, w_encoder, b_encoder, pad=1)           [100, 128, 128]
  Y = softmax over k of E.reshape(4, 25, H, W)          (s, k, h, w)
  out[s,c,h,w] = sum_k Y[s,k,h,w] * xpad[c, h+dy, w+dx] (k=(dy,dx), 5x5, pad 2)
  pixel-shuffle: out_ref[s*16 + c//4, 2h + (c//2)%2, 2w + c%2] = out[s,c,h,w]

Mapping (single pass, SBUF-resident, bf16 data / fp32 PSUM):
  - mask branch runs w-major in pipelined 16-column superblocks:
    conv1x1 -> relu -> R (zero-padded, w-major [49, 130*130]), conv3x3 as
    9 accumulating matmuls, exp on ScalarE into a small ring, then PE
    transposes write each column (value-duplicated) into pixel-major
    fr2 [h, (s,k,w,2)] holding UNNORMALIZED exp values.
  - the apply is split by w-half: the half-0 units only need fr2 columns
    from superblocks 0-3, so (with precise range dependencies) the DVE
    starts the apply while TensorE is still convolving superblocks 4-7.
    X^T copies are staged per half so both halves' working sets fit SBUF
    alongside the conv buffers.
  - softmax denominator: DVE strided reduce over k per w-half +
    reciprocal; normalization is folded into the apply epilogue.
  - apply on DVE in pixel-major [h, (c_hi, w, c_lo)] channel-pair layout
    (the duplicated pair keeps every operand 4B-aligned -> DVE 2x mode):
    per (s, half): 25 muls + 20 in-group adds (bf16, 2x) build 5
    dy-partials; partials are accumulated in fp32 PSUM by identity
    matmuls on TensorE; the DVE epilogue multiplies PSUM by recipZ
    (normalization) and rounds once to bf16.
  - pixel-shuffle falls out of the channel-pair layout: output DMA writes
    (w, c_lo) runs of 256 contiguous bf16 per (c4, row).
"""

import sys

import numpy as np

sys.path.insert(0, "/opt/trn_rl_repo")

import ml_dtypes

import concourse.bass as bass
import concourse.mybir as mybir
import concourse.tile as tile
from concourse import bacc
from concourse.masks import make_identity

F32 = mybir.dt.float32
BF16 = mybir.dt.bfloat16
BF_NP = ml_dtypes.bfloat16

H = 128
W = 128
C = 64
CH = 32  # channel pairs
M = 48  # compressed channels
S2 = 4  # scale_factor**2
K2 = 25  # k_up**2
SK = 100
HW = H * W
RP = 130  # padded R row pitch (w-major: row = w, col = h)
WQ = 132  # padded w for the interleaved X^T buffer
XF = CH * WQ * 2  # full xtq row: 8448 elements
WHQ = 70  # w-columns per half-staged X^T tile (64 + 2*3 halo/alignment)
XFH = CH * WHQ * 2  # 4480 elements per half tile
F2 = SK * W * 2  # 25600 free elements of fr2
NSB = 8  # superblocks (16 w-columns each)
SBW = W // NSB  # 16 w-columns per superblock
SBF = SBW * H  # 2048 pixels per superblock
WH = W // 2  # 64 output w per half
FH = CH * WH * 2  # 4096 free elements per (s, half) unit
N_CORES = 8


def _ap(t, extra_off, dims):
    """Raw AP on a tile handle `t` with free-offset `extra_off` (elements)
    and explicit [step, count] dims (dims[0] is the partition dim)."""
    base = t[:]
    return bass.AP(tensor=base.tensor, offset=base.offset + extra_off, ap=dims)


class _Pool:
    """Manually scoped tile pool."""

    def __init__(self, tc, **kw):
        self._cm = tc.tile_pool(**kw)
        self.pool = self._cm.__enter__()
        self._n = 0

    def tile(self, *a, tag=None, **kw):
        self._n += 1
        t = tag or f"t{self._n}"
        return self.pool.tile(*a, tag=t, name=t, **kw)

    def close(self):
        self._cm.__exit__(None, None, None)


def build_program(debug=False):
    nc = bacc.Bacc("TRN2", target_bir_lowering=False, debug=False)

    x_wm = nc.dram_tensor("x_wm", [C + 1, HW], BF16, kind="ExternalInput")
    w1t = nc.dram_tensor("w1t", [C + 1, M], BF16, kind="ExternalInput")
    wet = nc.dram_tensor("wet", [M + 1, 9 * SK], BF16, kind="ExternalInput")
    xtq = nc.dram_tensor("xtq", [WQ, XF], BF16, kind="ExternalInput")
    onesr = nc.dram_tensor("onesr", [1, RP * RP], BF16, kind="ExternalInput")
    out = nc.dram_tensor("out", [C, 2 * H, 2 * W], BF16, kind="ExternalOutput")
    dbg = {}
    if debug:
        dbg["FR2"] = nc.dram_tensor("dbgFR2", [128, F2], BF16, kind="ExternalOutput")
        dbg["Z"] = nc.dram_tensor("dbgZ", [128, S2 * W], F32, kind="ExternalOutput")

    with tile.TileContext(nc) as tc:
        cp = _Pool(tc, name="consts", bufs=1)
        w1t_sb = cp.tile([C + 1, M], BF16)
        nc.sync.dma_start(w1t_sb[:], w1t.ap())
        wet_sb = cp.tile([M + 1, 9 * SK], BF16)
        nc.sync.dma_start(wet_sb[:], wet.ap())
        ident = cp.tile([SK, SK], BF16)
        make_identity(nc, ident[:])
        id128 = cp.tile([128, 128], BF16)
        make_identity(nc, id128[:])

        fr2p = _Pool(tc, name="fr2", bufs=1)
        fr2 = fr2p.tile([128, F2], BF16)
        zp = _Pool(tc, name="z", bufs=1)
        zbuf = zp.tile([128, S2 * W], F32)

        def load_xtd(pool, half):
            """5 dy-shifted, half-w X^T tiles [128, CH*70*2]."""
            tiles = []
            for i in range(5):  # dy = i - 2
                td = pool.tile([128, XFH], BF16, tag=f"xtd{half}_{i}")
                nc.sync.dma_start(
                    td[:],
                    bass.AP(
                        tensor=xtq,
                        offset=i * XF + half * (WH - 2) * 2,
                        ap=[[XF, 128], [WQ * 2, CH], [1, WHQ * 2]],
                    ),
                )
                tiles.append(td)
            return tiles

        xh0p = _Pool(tc, name="xth0", bufs=1)
        xtd0 = load_xtd(xh0p, 0)

        # ---- mask branch, pipelined w-major superblocks ----
        rp_ = _Pool(tc, name="R", bufs=1)
        R = rp_.tile([M + 1, RP * RP], BF16)
        nc.gpsimd.memset(R[:], 0.0)
        nc.sync.dma_start(
            _ap(R, M * RP * RP, [[RP * RP, 1], [1, RP * RP]]), onesr.ap()
        )
        xrp = _Pool(tc, name="xring", bufs=2)
        frg = _Pool(tc, name="fring", bufs=2)
        psA = _Pool(tc, name="psA", bufs=2, space="PSUM")
        psB = _Pool(tc, name="psB", bufs=1, space="PSUM")
        psF = _Pool(tc, name="psF", bufs=2, space="PSUM")

        def conv1x1_sb(sb):
            """conv1x1 + relu for superblock sb -> R columns (w-major)."""
            xs = xrp.tile([C + 1, SBF], BF16, tag="xs")
            nc.sync.dma_start(xs[:], x_wm.ap()[:, sb * SBF : (sb + 1) * SBF])
            for j in range(4):  # 4 chunks of 4 w-cols
                ps = psA.tile([M, 512], F32, tag="ps1")
                nc.tensor.matmul(
                    ps[:], w1t_sb[:], xs[:, j * 512 : (j + 1) * 512],
                    start=True, stop=True,
                )
                w0 = sb * SBW + 4 * j
                nc.scalar.activation(
                    _ap(R, (1 + w0) * RP + 1, [[RP * RP, M], [RP, 4], [1, H]]),
                    ps[:],
                    mybir.ActivationFunctionType.Relu,
                )

        def conv3x3_sb(sb):
            """conv3x3 + exp + transposes for superblock sb -> fr2 columns."""
            fs = frg.tile([SK, SBF], BF16, tag="fs")
            # tap-outer over the superblock's 4 chunks (4 PSUM banks)
            pss = [psB.tile([SK, 512], F32, tag=f"ps2_{j}") for j in range(4)]
            for t in range(9):
                ty, tx = divmod(t, 3)
                for j in range(4):
                    nc.tensor.matmul(
                        pss[j][:],
                        wet_sb[:, t * SK : (t + 1) * SK],
                        _ap(R, (sb * SBW + 4 * j + tx) * RP + ty,
                            [[RP * RP, M + 1], [RP, 4], [1, H]]),
                        start=(t == 0),
                        stop=(t == 8),
                    )
            for j in range(4):
                nc.scalar.activation(
                    fs[:, j * 512 : (j + 1) * 512],
                    pss[j][:],
                    mybir.ActivationFunctionType.Exp,
                )
            for wi in range(SBW):
                pst = psF.tile([128, SK], BF16, tag="pst")
                nc.tensor.transpose(
                    pst[:], fs[:, wi * H : (wi + 1) * H], ident[:]
                )
                nc.scalar.copy(
                    _ap(fr2, (sb * SBW + wi) * 2, [[F2, 128], [W * 2, SK], [1, 2]]),
                    _ap(pst, 0, [[SK, 128], [1, SK], [0, 2]]),
                )

        def z_half(half):
            """softmax denominator + reciprocal for w-columns of `half`."""
            for s in range(S2):
                nc.vector.reduce_sum(
                    zbuf[:, s * W + half * WH : s * W + (half + 1) * WH],
                    _ap(fr2, (s * K2 * W + half * WH) * 2,
                        [[F2, 128], [2, WH], [W * 2, K2]]),
                    axis=mybir.AxisListType.X,
                )
            nc.vector.reciprocal(
                _ap(zbuf, half * WH, [[S2 * W, 128], [W, S2], [1, WH]]),
                _ap(zbuf, half * WH, [[S2 * W, 128], [W, S2], [1, WH]]),
            )

        GROUPS5 = [list(range(5 * i, 5 * i + 5)) for i in range(5)]
        GROUPS2 = [[2 * i, 2 * i + 1] for i in range(12)] + [[24]]

        def apply_units(half, xtd, pp_, tp_, abfp, psc, groups_of):
            """4 (s, half) apply units: DVE taps, PE partial-merges,
            normalizing DVE epilogue, pixel-shuffle DMA out."""
            dims_u = [[FH, 128], [WH * 2, CH], [2, WH], [1, 2]]
            base_dx = 0 if half == 0 else 2
            for s in range(S2):
                groups = groups_of(s)
                psacc = psc.tile([128, FH], F32, tag="psacc")
                for gi, grp in enumerate(groups):
                    p = pp_.tile([128, FH], BF16, tag="p")
                    for n, k in enumerate(grp):
                        dyi, dxi = divmod(k, 5)
                        in0 = _ap(
                            xtd[dyi], (base_dx + dxi) * 2,
                            [[XFH, 128], [WHQ * 2, CH], [2, WH], [1, 2]],
                        )
                        in1 = _ap(
                            fr2, ((s * K2 + k) * W + half * WH) * 2,
                            [[F2, 128], [0, CH], [2, WH], [1, 2]],
                        )
                        if n == 0:
                            nc.vector.tensor_mul(_ap(p, 0, dims_u), in0, in1)
                        else:
                            t = tp_.tile([128, FH], BF16, tag="t")
                            nc.vector.tensor_mul(_ap(t, 0, dims_u), in0, in1)
                            nc.vector.tensor_add(p[:], p[:], t[:])
                    # accumulate the partial into fp32 PSUM on TensorE
                    for sl in range(8):
                        nc.tensor.matmul(
                            psacc[:, sl * 512 : (sl + 1) * 512],
                            id128[:],
                            p[:, sl * 512 : (sl + 1) * 512],
                            start=(gi == 0),
                            stop=(gi == len(groups) - 1),
                            skip_group_check=True,
                        )
                # normalize + round once: accbf = psacc * recipZ
                accbf = abfp.tile([128, FH], BF16, tag="accbf")
                nc.vector.tensor_mul(
                    _ap(accbf, 0, dims_u),
                    _ap(psacc, 0, dims_u),
                    _ap(zbuf, s * W + half * WH,
                        [[S2 * W, 128], [0, CH], [1, WH], [0, 2]]),
                )
                # pixel-shuffle output: c = 2*c_hi + c_lo; c4 = c_hi//2,
                # c2 = c_hi%2, c1 = c_lo; rows 2h+c2, cols (2w+c1)
                for c2 in range(2):
                    dst = bass.AP(
                        tensor=out,
                        offset=(s * 16) * (4 * HW) + c2 * (2 * W) + half * (2 * WH),
                        ap=[[2 * (2 * W), 128], [4 * HW, 16], [1, 2 * WH]],
                    )
                    src = _ap(
                        accbf, c2 * (WH * 2),
                        [[FH, 128], [2 * (WH * 2), 16], [1, 2 * WH]],
                    )
                    nc.sync.dma_start(dst, src)

        # conv pipeline superblocks 0-3; Zh0 emitted as soon as its fr2
        # columns exist so its range dependencies stay tight
        conv1x1_sb(0)
        for sb in range(1, 5):
            conv1x1_sb(sb)
            conv3x3_sb(sb - 1)
        z_half(0)
        for sb in range(5, NSB):
            conv1x1_sb(sb)
            conv3x3_sb(sb - 1)
        conv3x3_sb(NSB - 1)
        psF.close()
        psB.close()
        psA.close()

        # half 0 apply overlaps the tail of the conv pipeline; the first
        # unit uses 5-tap groups (fewer PE merges while TensorE is still
        # convolving), later units use pair-groups to offload adds to PE
        pp_ = _Pool(tc, name="pbuf", bufs=3)
        tp_ = _Pool(tc, name="tbuf", bufs=1)
        abfp = _Pool(tc, name="accbf", bufs=2)
        psc = _Pool(tc, name="psacc", bufs=1, space="PSUM")
        apply_units(0, xtd0, pp_, tp_, abfp, psc,
                    lambda s: GROUPS5 if s == 0 else GROUPS2)
        psc.close()
        abfp.close()
        tp_.close()
        pp_.close()
        frg.close()
        xrp.close()
        rp_.close()
        xh0p.close()

        # half 1: X^T half tiles stream in while half 0 finishes
        xh1p = _Pool(tc, name="xth1", bufs=1)
        xtd1 = load_xtd(xh1p, 1)
        pp2 = _Pool(tc, name="pbuf2", bufs=3)
        tp2 = _Pool(tc, name="tbuf2", bufs=1)
        abf2 = _Pool(tc, name="accbf2", bufs=2)
        psc2 = _Pool(tc, name="psacc2", bufs=1, space="PSUM")
        z_half(1)
        apply_units(1, xtd1, pp2, tp2, abf2, psc2, lambda s: GROUPS2)
        psc2.close()
        abf2.close()
        tp2.close()
        pp2.close()
        xh1p.close()

        if debug:
            nc.sync.dma_start(dbg["FR2"].ap(), fr2[:])
            nc.sync.dma_start(dbg["Z"].ap(), zbuf[:])
        zp.close()
        fr2p.close()
        cp.close()
    nc.compile()
    return nc, dbg


def host_inputs(x_img, w_compress, b_compress, w_encoder, b_encoder):
    """Per-core input map for one image [C, H, W] (all bf16)."""
    w1t = np.concatenate(
        [w_compress[:, :, 0, 0].T, b_compress[None, :]], axis=0
    ).astype(BF_NP)
    wet = np.zeros((M + 1, 9, SK), np.float32)
    for ty in range(3):
        for tx in range(3):
            wet[:M, ty * 3 + tx, :] = w_encoder[:, :, ty, tx].T
    wet[M, 4, :] = b_encoder
    x_wm = np.ones((C + 1, HW), np.float32)
    x_wm[:C] = x_img.transpose(0, 2, 1).reshape(C, HW)  # w-major
    # xtq[j, c_hi, wq, c_lo] = xpad[2*c_hi + c_lo, j-2, wq-2]
    xtq = np.zeros((WQ, CH, WQ, 2), np.float32)
    xtq[2:130, :, 2:130, :] = (
        x_img.reshape(CH, 2, H, W).transpose(2, 0, 3, 1)
    )
    return {
        "x_wm": x_wm.astype(BF_NP),
        "w1t": w1t,
        "wet": wet.reshape(M + 1, 9 * SK).astype(BF_NP),
        "xtq": xtq.reshape(WQ, XF).astype(BF_NP),
        "onesr": np.ones((1, RP * RP), BF_NP),
    }


_CACHE = {}


def kernel(x, w_compress, b_compress, w_encoder, b_encoder):
    x = np.asarray(x, np.float32)
    if "nc" not in _CACHE:
        _CACHE["nc"], _ = build_program(debug=False)
    nc = _CACHE["nc"]
    in_maps = [
        host_inputs(
            x[i],
            np.asarray(w_compress, np.float32),
            np.asarray(b_compress, np.float32),
            np.asarray(w_encoder, np.float32),
            np.asarray(b_encoder, np.float32),
        )
        for i in range(N_CORES)
    ]
    from concourse.bass_utils import run_bass_kernel_spmd

    res = run_bass_kernel_spmd(nc, in_maps, core_ids=list(range(N_CORES)))
    return np.stack(
        [res.results[i]["out"].astype(np.float32) for i in range(N_CORES)], axis=0
    )


# revision 27
# speedup vs baseline: 1.4510x; 1.0053x over previous
"""CARAFE kernel for Trainium2 (8 NeuronCores, batch-parallel), bf16 pipeline.

Reference computation per image:
  R = relu(conv1x1(x, w_compress, b_compress))          [48, 128, 128]
  E = conv3x3(R, w_encoder, b_encoder, pad=1)           [100, 128, 128]
  Y = softmax over k of E.reshape(4, 25, H, W)          (s, k, h, w)
  out[s,c,h,w] = sum_k Y[s,k,h,w] * xpad[c, h+dy, w+dx] (k=(dy,dx), 5x5, pad 2)
  pixel-shuffle: out_ref[s*16 + c//4, 2h + (c//2)%2, 2w + c%2] = out[s,c,h,w]

Mapping (single pass, SBUF-resident, bf16 data / fp32 PSUM):
  - mask branch runs w-major in pipelined 16-column superblocks:
    conv1x1 -> relu -> R (zero-padded, w-major [49, 130*130]), conv3x3 as
    9 accumulating matmuls, exp on ScalarE into a small ring, then PE
    transposes write each column (value-duplicated) into pixel-major
    fr2 [h, (s,k,w,2)] holding UNNORMALIZED exp values.
  - the apply is split by w-half: the half-0 units only need fr2 columns
    from superblocks 0-3, so (with precise range dependencies) the DVE
    starts the apply while TensorE is still convolving superblocks 4-7.
    X^T copies are staged per half so both halves' working sets fit SBUF
    alongside the conv buffers.
  - softmax denominator: DVE strided reduce over k per w-half +
    reciprocal; normalization is folded into the apply epilogue.
  - apply on DVE in pixel-major [h, (c_hi, w, c_lo)] channel-pair layout
    (the duplicated pair keeps every operand 4B-aligned -> DVE 2x mode):
    per (s, half): 25 muls + 20 in-group adds (bf16, 2x) build 5
    dy-partials; partials are accumulated in fp32 PSUM by identity
    matmuls on TensorE; the DVE epilogue multiplies PSUM by recipZ
    (normalization) and rounds once to bf16.
  - pixel-shuffle falls out of the channel-pair layout: output DMA writes
    (w, c_lo) runs of 256 contiguous bf16 per (c4, row).
"""

import sys

import numpy as np

sys.path.insert(0, "/opt/trn_rl_repo")

import ml_dtypes

import concourse.bass as bass
import concourse.mybir as mybir
import concourse.tile as tile
from concourse import bacc
from concourse.masks import make_identity

F32 = mybir.dt.float32
BF16 = mybir.dt.bfloat16
BF_NP = ml_dtypes.bfloat16

H = 128
W = 128
C = 64
CH = 32  # channel pairs
M = 48  # compressed channels
S2 = 4  # scale_factor**2
K2 = 25  # k_up**2
SK = 100
HW = H * W
RP = 130  # padded R row pitch (w-major: row = w, col = h)
WQ = 132  # padded w for the interleaved X^T buffer
XF = CH * WQ * 2  # full xtq row: 8448 elements
WHQ = 70  # w-columns per half-staged X^T tile (64 + 2*3 halo/alignment)
XFH = CH * WHQ * 2  # 4480 elements per half tile
F2 = SK * W * 2  # 25600 free elements of fr2
NSB = 8  # superblocks (16 w-columns each)
SBW = W // NSB  # 16 w-columns per superblock
SBF = SBW * H  # 2048 pixels per superblock
WH = W // 2  # 64 output w per half
FH = CH * WH * 2  # 4096 free elements per (s, half) unit
N_CORES = 8


def _ap(t, extra_off, dims):
    """Raw AP on a tile handle `t` with free-offset `extra_off` (elements)
    and explicit [step, count] dims (dims[0] is the partition dim)."""
    base = t[:]
    return bass.AP(tensor=base.tensor, offset=base.offset + extra_off, ap=dims)


class _Pool:
    """Manually scoped tile pool."""

    def __init__(self, tc, **kw):
        self._cm = tc.tile_pool(**kw)
        self.pool = self._cm.__enter__()
        self._n = 0

    def tile(self, *a, tag=None, **kw):
        self._n += 1
        t = tag or f"t{self._n}"
        return self.pool.tile(*a, tag=t, name=t, **kw)

    def close(self):
        self._cm.__exit__(None, None, None)


def build_program(debug=False):
    nc = bacc.Bacc("TRN2", target_bir_lowering=False, debug=False)

    x_wm = nc.dram_tensor("x_wm", [C + 1, HW], BF16, kind="ExternalInput")
    w1t = nc.dram_tensor("w1t", [C + 1, M], BF16, kind="ExternalInput")
    wet = nc.dram_tensor("wet", [M + 1, 9 * SK], BF16, kind="ExternalInput")
    xtq = nc.dram_tensor("xtq", [WQ, XF], BF16, kind="ExternalInput")
    onesr = nc.dram_tensor("onesr", [1, RP * RP], BF16, kind="ExternalInput")
    out = nc.dram_tensor("out", [C, 2 * H, 2 * W], BF16, kind="ExternalOutput")
    dbg = {}
    if debug:
        dbg["FR2"] = nc.dram_tensor("dbgFR2", [128, F2], BF16, kind="ExternalOutput")
        dbg["Z"] = nc.dram_tensor("dbgZ", [128, S2 * W], F32, kind="ExternalOutput")

    with tile.TileContext(nc) as tc:
        cp = _Pool(tc, name="consts", bufs=1)
        w1t_sb = cp.tile([C + 1, M], BF16)
        nc.sync.dma_start(w1t_sb[:], w1t.ap())
        wet_sb = cp.tile([M + 1, 9 * SK], BF16)
        nc.sync.dma_start(wet_sb[:], wet.ap())
        ident = cp.tile([SK, SK], BF16)
        make_identity(nc, ident[:])
        id128 = cp.tile([128, 128], BF16)
        make_identity(nc, id128[:])

        fr2p = _Pool(tc, name="fr2", bufs=1)
        fr2 = fr2p.tile([128, F2], BF16)
        zp = _Pool(tc, name="z", bufs=1)
        zbuf = zp.tile([128, S2 * W], F32)

        def load_xtd(pool, half):
            """5 dy-shifted, half-w X^T tiles [128, CH*70*2]."""
            tiles = []
            for i in range(5):  # dy = i - 2
                td = pool.tile([128, XFH], BF16, tag=f"xtd{half}_{i}")
                nc.sync.dma_start(
                    td[:],
                    bass.AP(
                        tensor=xtq,
                        offset=i * XF + half * (WH - 2) * 2,
                        ap=[[XF, 128], [WQ * 2, CH], [1, WHQ * 2]],
                    ),
                )
                tiles.append(td)
            return tiles

        xh0p = _Pool(tc, name="xth0", bufs=1)
        xtd0 = load_xtd(xh0p, 0)

        # ---- mask branch, pipelined w-major superblocks ----
        rp_ = _Pool(tc, name="R", bufs=1)
        R = rp_.tile([M + 1, RP * RP], BF16)
        nc.gpsimd.memset(R[:], 0.0)
        nc.sync.dma_start(
            _ap(R, M * RP * RP, [[RP * RP, 1], [1, RP * RP]]), onesr.ap()
        )
        xrp = _Pool(tc, name="xring", bufs=2)
        frg = _Pool(tc, name="fring", bufs=2)
        psA = _Pool(tc, name="psA", bufs=2, space="PSUM")
        psB = _Pool(tc, name="psB", bufs=1, space="PSUM")
        psF = _Pool(tc, name="psF", bufs=2, space="PSUM")

        def conv1x1_sb(sb):
            """conv1x1 + relu for superblock sb -> R columns (w-major)."""
            xs = xrp.tile([C + 1, SBF], BF16, tag="xs")
            nc.sync.dma_start(xs[:], x_wm.ap()[:, sb * SBF : (sb + 1) * SBF])
            for j in range(4):  # 4 chunks of 4 w-cols
                ps = psA.tile([M, 512], F32, tag="ps1")
                nc.tensor.matmul(
                    ps[:], w1t_sb[:], xs[:, j * 512 : (j + 1) * 512],
                    start=True, stop=True,
                )
                w0 = sb * SBW + 4 * j
                nc.scalar.activation(
                    _ap(R, (1 + w0) * RP + 1, [[RP * RP, M], [RP, 4], [1, H]]),
                    ps[:],
                    mybir.ActivationFunctionType.Relu,
                )

        def conv3x3_sb(sb):
            """conv3x3 + exp + transposes for superblock sb -> fr2 columns."""
            fs = frg.tile([SK, SBF], BF16, tag="fs")
            # tap-outer over the superblock's 4 chunks (4 PSUM banks)
            pss = [psB.tile([SK, 512], F32, tag=f"ps2_{j}") for j in range(4)]
            for t in range(9):
                ty, tx = divmod(t, 3)
                for j in range(4):
                    nc.tensor.matmul(
                        pss[j][:],
                        wet_sb[:, t * SK : (t + 1) * SK],
                        _ap(R, (sb * SBW + 4 * j + tx) * RP + ty,
                            [[RP * RP, M + 1], [RP, 4], [1, H]]),
                        start=(t == 0),
                        stop=(t == 8),
                    )
            for j in range(4):
                nc.scalar.activation(
                    fs[:, j * 512 : (j + 1) * 512],
                    pss[j][:],
                    mybir.ActivationFunctionType.Exp,
                )
            for wi in range(SBW):
                pst = psF.tile([128, SK], BF16, tag="pst")
                nc.tensor.transpose(
                    pst[:], fs[:, wi * H : (wi + 1) * H], ident[:]
                )
                nc.scalar.copy(
                    _ap(fr2, (sb * SBW + wi) * 2, [[F2, 128], [W * 2, SK], [1, 2]]),
                    _ap(pst, 0, [[SK, 128], [1, SK], [0, 2]]),
                )

        def z_half(half):
            """softmax denominator + reciprocal for w-columns of `half`."""
            for s in range(S2):
                nc.vector.reduce_sum(
                    zbuf[:, s * W + half * WH : s * W + (half + 1) * WH],
                    _ap(fr2, (s * K2 * W + half * WH) * 2,
                        [[F2, 128], [2, WH], [W * 2, K2]]),
                    axis=mybir.AxisListType.X,
                )
            nc.vector.reciprocal(
                _ap(zbuf, half * WH, [[S2 * W, 128], [W, S2], [1, WH]]),
                _ap(zbuf, half * WH, [[S2 * W, 128], [W, S2], [1, WH]]),
            )

        GROUPS5 = [list(range(5 * i, 5 * i + 5)) for i in range(5)]
        # 9 pairs + 7 singles: balances DVE in-group adds vs PE merge rate;
        # interleaved so the p-ring never outruns the PE merge of ring-buf n-2
        _pairs = [[2 * i, 2 * i + 1] for i in range(9)]
        _singles = [[k] for k in range(18, 25)]
        GROUPS16 = []
        for i in range(9):
            GROUPS16.append(_pairs[i])
            if i < 7:
                GROUPS16.append(_singles[i])

        def apply_units(half, xtd, pp_, tp_, abfp, psc, groups_of, a32p=None):
            """4 (s, half) apply units: DVE taps, partial-merges on PE
            (fp32 PSUM) or on DVE (fp32 SBUF) while PE is busy convolving,
            normalizing DVE epilogue, pixel-shuffle DMA out."""
            dims_u = [[FH, 128], [WH * 2, CH], [2, WH], [1, 2]]
            base_dx = 0 if half == 0 else 2
            for s in range(S2):
                groups, dve_merge = groups_of(s)
                acc = (
                    a32p.tile([128, FH], F32, tag="acc32")
                    if dve_merge
                    else psc.tile([128, FH], F32, tag="psacc")
                )
                for gi, grp in enumerate(groups):
                    p = pp_.tile([128, FH], BF16, tag="p")
                    last = len(grp) - 1
                    for n, k in enumerate(grp):
                        dyi, dxi = divmod(k, 5)
                        in0 = _ap(
                            xtd[dyi], (base_dx + dxi) * 2,
                            [[XFH, 128], [WHQ * 2, CH], [2, WH], [1, 2]],
                        )
                        in1 = _ap(
                            fr2, ((s * K2 + k) * W + half * WH) * 2,
                            [[F2, 128], [0, CH], [2, WH], [1, 2]],
                        )
                        if n == 0:
                            nc.vector.tensor_mul(_ap(p, 0, dims_u), in0, in1)
                        else:
                            t = tp_.tile([128, FH], BF16, tag="t")
                            nc.vector.tensor_mul(_ap(t, 0, dims_u), in0, in1)
                            if dve_merge and gi == 0 and n == last:
                                nc.vector.tensor_add(acc[:], p[:], t[:])
                            else:
                                nc.vector.tensor_add(p[:], p[:], t[:])
                    if dve_merge:
                        if gi > 0:
                            nc.vector.tensor_add(acc[:], acc[:], p[:])
                    else:
                        # accumulate the partial into fp32 PSUM on TensorE
                        for sl in range(8):
                            nc.tensor.matmul(
                                acc[:, sl * 512 : (sl + 1) * 512],
                                id128[:],
                                p[:, sl * 512 : (sl + 1) * 512],
                                start=(gi == 0),
                                stop=(gi == len(groups) - 1),
                                skip_group_check=True,
                            )
                # normalize + round once: accbf = acc * recipZ
                accbf = abfp.tile([128, FH], BF16, tag="accbf")
                nc.vector.tensor_mul(
                    _ap(accbf, 0, dims_u),
                    _ap(acc, 0, dims_u),
                    _ap(zbuf, s * W + half * WH,
                        [[S2 * W, 128], [0, CH], [1, WH], [0, 2]]),
                )
                # pixel-shuffle output: c = 2*c_hi + c_lo; c4 = c_hi//2,
                # c2 = c_hi%2, c1 = c_lo; rows 2h+c2, cols (2w+c1)
                for c2 in range(2):
                    dst = bass.AP(
                        tensor=out,
                        offset=(s * 16) * (4 * HW) + c2 * (2 * W) + half * (2 * WH),
                        ap=[[2 * (2 * W), 128], [4 * HW, 16], [1, 2 * WH]],
                    )
                    src = _ap(
                        accbf, c2 * (WH * 2),
                        [[FH, 128], [2 * (WH * 2), 16], [1, 2 * WH]],
                    )
                    nc.sync.dma_start(dst, src)

        # conv pipeline superblocks 0-3; Zh0 emitted as soon as its fr2
        # columns exist so its range dependencies stay tight
        conv1x1_sb(0)
        for sb in range(1, 5):
            conv1x1_sb(sb)
            conv3x3_sb(sb - 1)
        z_half(0)
        for sb in range(5, NSB):
            conv1x1_sb(sb)
            conv3x3_sb(sb - 1)
        conv3x3_sb(NSB - 1)
        psF.close()
        psB.close()
        psA.close()

        # half 0 apply overlaps the tail of the conv pipeline; the first
        # unit merges its 5-tap partials on the DVE (TensorE is still
        # convolving), later units offload merges to PE via small groups
        pp_ = _Pool(tc, name="pbuf", bufs=2)
        tp_ = _Pool(tc, name="tbuf", bufs=1)
        abfp = _Pool(tc, name="accbf", bufs=1)
        a32p = _Pool(tc, name="acc32", bufs=1)
        psc = _Pool(tc, name="psacc", bufs=1, space="PSUM")
        apply_units(0, xtd0, pp_, tp_, abfp, psc,
                    lambda s: (GROUPS5, True) if s == 0 else (GROUPS16, False),
                    a32p=a32p)
        psc.close()
        a32p.close()
        abfp.close()
        tp_.close()
        pp_.close()
        frg.close()
        xrp.close()
        rp_.close()
        xh0p.close()

        # half 1: X^T half tiles stream in while half 0 finishes
        xh1p = _Pool(tc, name="xth1", bufs=1)
        xtd1 = load_xtd(xh1p, 1)
        pp2 = _Pool(tc, name="pbuf2", bufs=3)
        tp2 = _Pool(tc, name="tbuf2", bufs=1)
        abf2 = _Pool(tc, name="accbf2", bufs=2)
        psc2 = _Pool(tc, name="psacc2", bufs=1, space="PSUM")
        z_half(1)
        apply_units(1, xtd1, pp2, tp2, abf2, psc2, lambda s: (GROUPS16, False))
        psc2.close()
        abf2.close()
        tp2.close()
        pp2.close()
        xh1p.close()

        if debug:
            nc.sync.dma_start(dbg["FR2"].ap(), fr2[:])
            nc.sync.dma_start(dbg["Z"].ap(), zbuf[:])
        zp.close()
        fr2p.close()
        cp.close()
    nc.compile()
    return nc, dbg


def host_inputs(x_img, w_compress, b_compress, w_encoder, b_encoder):
    """Per-core input map for one image [C, H, W] (all bf16)."""
    w1t = np.concatenate(
        [w_compress[:, :, 0, 0].T, b_compress[None, :]], axis=0
    ).astype(BF_NP)
    wet = np.zeros((M + 1, 9, SK), np.float32)
    for ty in range(3):
        for tx in range(3):
            wet[:M, ty * 3 + tx, :] = w_encoder[:, :, ty, tx].T
    wet[M, 4, :] = b_encoder
    x_wm = np.ones((C + 1, HW), np.float32)
    x_wm[:C] = x_img.transpose(0, 2, 1).reshape(C, HW)  # w-major
    # xtq[j, c_hi, wq, c_lo] = xpad[2*c_hi + c_lo, j-2, wq-2]
    xtq = np.zeros((WQ, CH, WQ, 2), np.float32)
    xtq[2:130, :, 2:130, :] = (
        x_img.reshape(CH, 2, H, W).transpose(2, 0, 3, 1)
    )
    return {
        "x_wm": x_wm.astype(BF_NP),
        "w1t": w1t,
        "wet": wet.reshape(M + 1, 9 * SK).astype(BF_NP),
        "xtq": xtq.reshape(WQ, XF).astype(BF_NP),
        "onesr": np.ones((1, RP * RP), BF_NP),
    }


_CACHE = {}


def kernel(x, w_compress, b_compress, w_encoder, b_encoder):
    x = np.asarray(x, np.float32)
    if "nc" not in _CACHE:
        _CACHE["nc"], _ = build_program(debug=False)
    nc = _CACHE["nc"]
    in_maps = [
        host_inputs(
            x[i],
            np.asarray(w_compress, np.float32),
            np.asarray(b_compress, np.float32),
            np.asarray(w_encoder, np.float32),
            np.asarray(b_encoder, np.float32),
        )
        for i in range(N_CORES)
    ]
    from concourse.bass_utils import run_bass_kernel_spmd

    res = run_bass_kernel_spmd(nc, in_maps, core_ids=list(range(N_CORES)))
    return np.stack(
        [res.results[i]["out"].astype(np.float32) for i in range(N_CORES)], axis=0
    )


# revision 28
# speedup vs baseline: 1.5028x; 1.0357x over previous
"""CARAFE kernel for Trainium2 (8 NeuronCores, batch-parallel), bf16 pipeline.

Reference computation per image:
  R = relu(conv1x1(x, w_compress, b_compress))          [48, 128, 128]
  E = conv3x3(R, w_encoder, b_encoder, pad=1)           [100, 128, 128]
  Y = softmax over k of E.reshape(4, 25, H, W)          (s, k, h, w)
  out[s,c,h,w] = sum_k Y[s,k,h,w] * xpad[c, h+dy, w+dx] (k=(dy,dx), 5x5, pad 2)
  pixel-shuffle: out_ref[s*16 + c//4, 2h + (c//2)%2, 2w + c%2] = out[s,c,h,w]

Mapping (single pass, SBUF-resident, bf16 data / fp32 PSUM):
  - mask branch runs w-major in pipelined 16-column superblocks:
    conv1x1 -> relu -> R (zero-padded, w-major [49, 130*130]), conv3x3 as
    9 accumulating matmuls, exp on ScalarE into a small ring, then PE
    transposes write each column (value-duplicated) into pixel-major
    fr2 [h, (s,k,w,2)] holding UNNORMALIZED exp values.
  - the apply is split by w-half: the half-0 units only need fr2 columns
    from superblocks 0-3, so (with precise range dependencies) the DVE
    starts the apply while TensorE is still convolving superblocks 4-7.
    X^T copies are staged per half so both halves' working sets fit SBUF
    alongside the conv buffers.
  - softmax denominator: DVE strided reduce over k per w-half +
    reciprocal; normalization is folded into the apply epilogue.
  - apply on DVE in pixel-major [h, (c_hi, w, c_lo)] channel-pair layout
    (the duplicated pair keeps every operand 4B-aligned -> DVE 2x mode):
    per (s, half): 25 muls + 20 in-group adds (bf16, 2x) build 5
    dy-partials; partials are accumulated in fp32 PSUM by identity
    matmuls on TensorE; the DVE epilogue multiplies PSUM by recipZ
    (normalization) and rounds once to bf16.
  - pixel-shuffle falls out of the channel-pair layout: output DMA writes
    (w, c_lo) runs of 256 contiguous bf16 per (c4, row).
"""

import sys

import numpy as np

sys.path.insert(0, "/opt/trn_rl_repo")

import ml_dtypes

import concourse.bass as bass
import concourse.mybir as mybir
import concourse.tile as tile
from concourse import bacc
from concourse.masks import make_identity

F32 = mybir.dt.float32
BF16 = mybir.dt.bfloat16
BF_NP = ml_dtypes.bfloat16

H = 128
W = 128
C = 64
CH = 32  # channel pairs
M = 48  # compressed channels
S2 = 4  # scale_factor**2
K2 = 25  # k_up**2
SK = 100
HW = H * W
RP = 130  # padded R row pitch (w-major: row = w, col = h)
WQ = 132  # padded w for the interleaved X^T buffer
XF = CH * WQ * 2  # full xtq row: 8448 elements
WHQ = 70  # w-columns per half-staged X^T tile (64 + 2*3 halo/alignment)
XFH = CH * WHQ * 2  # 4480 elements per half tile
F2 = SK * W * 2  # 25600 free elements of fr2
NSB = 8  # superblocks (16 w-columns each)
SBW = W // NSB  # 16 w-columns per superblock
SBF = SBW * H  # 2048 pixels per superblock
WH = W // 2  # 64 output w per half
FH = CH * WH * 2  # 4096 free elements per (s, half) unit
N_CORES = 8


def _ap(t, extra_off, dims):
    """Raw AP on a tile handle `t` with free-offset `extra_off` (elements)
    and explicit [step, count] dims (dims[0] is the partition dim)."""
    base = t[:]
    return bass.AP(tensor=base.tensor, offset=base.offset + extra_off, ap=dims)


class _Pool:
    """Manually scoped tile pool."""

    def __init__(self, tc, **kw):
        self._cm = tc.tile_pool(**kw)
        self.pool = self._cm.__enter__()
        self._n = 0

    def tile(self, *a, tag=None, **kw):
        self._n += 1
        t = tag or f"t{self._n}"
        return self.pool.tile(*a, tag=t, name=t, **kw)

    def close(self):
        self._cm.__exit__(None, None, None)


def build_program(debug=False):
    nc = bacc.Bacc("TRN2", target_bir_lowering=False, debug=False)

    x_wm = nc.dram_tensor("x_wm", [C + 1, HW], BF16, kind="ExternalInput")
    w1t = nc.dram_tensor("w1t", [C + 1, M], BF16, kind="ExternalInput")
    wet = nc.dram_tensor("wet", [M + 1, 9 * SK], BF16, kind="ExternalInput")
    xtq = nc.dram_tensor("xtq", [WQ, XF], BF16, kind="ExternalInput")
    onesr = nc.dram_tensor("onesr", [1, RP * RP], BF16, kind="ExternalInput")
    out = nc.dram_tensor("out", [C, 2 * H, 2 * W], BF16, kind="ExternalOutput")
    dbg = {}
    if debug:
        dbg["FR2"] = nc.dram_tensor("dbgFR2", [128, F2], BF16, kind="ExternalOutput")
        dbg["Z"] = nc.dram_tensor("dbgZ", [128, S2 * W], F32, kind="ExternalOutput")

    with tile.TileContext(nc) as tc:
        cp = _Pool(tc, name="consts", bufs=1)
        w1t_sb = cp.tile([C + 1, M], BF16)
        nc.sync.dma_start(w1t_sb[:], w1t.ap())
        wet_sb = cp.tile([M + 1, 9 * SK], BF16)
        nc.sync.dma_start(wet_sb[:], wet.ap())
        ident = cp.tile([SK, SK], BF16)
        make_identity(nc, ident[:])
        id128 = cp.tile([128, 128], BF16)
        make_identity(nc, id128[:])

        fr2p = _Pool(tc, name="fr2", bufs=1)
        fr2 = fr2p.tile([128, F2], BF16)
        zp = _Pool(tc, name="z", bufs=1)
        zbuf = zp.tile([128, S2 * W], F32)

        def load_xtd(pool, half):
            """5 dy-shifted, half-w X^T tiles [128, CH*70*2]."""
            tiles = []
            for i in range(5):  # dy = i - 2
                td = pool.tile([128, XFH], BF16, tag=f"xtd{half}_{i}")
                nc.sync.dma_start(
                    td[:],
                    bass.AP(
                        tensor=xtq,
                        offset=i * XF + half * (WH - 2) * 2,
                        ap=[[XF, 128], [WQ * 2, CH], [1, WHQ * 2]],
                    ),
                )
                tiles.append(td)
            return tiles

        xh0p = _Pool(tc, name="xth0", bufs=1)
        xtd0 = load_xtd(xh0p, 0)

        # ---- mask branch, pipelined w-major superblocks ----
        rp_ = _Pool(tc, name="R", bufs=1)
        R = rp_.tile([M + 1, RP * RP], BF16)
        nc.gpsimd.memset(R[:], 0.0)
        nc.sync.dma_start(
            _ap(R, M * RP * RP, [[RP * RP, 1], [1, RP * RP]]), onesr.ap()
        )
        xrp = _Pool(tc, name="xring", bufs=2)
        frg = _Pool(tc, name="fring", bufs=2)
        psA = _Pool(tc, name="psA", bufs=2, space="PSUM")
        psB = _Pool(tc, name="psB", bufs=1, space="PSUM")
        psF = _Pool(tc, name="psF", bufs=2, space="PSUM")

        def conv1x1_sb(sb):
            """conv1x1 + relu for superblock sb -> R columns (w-major)."""
            xs = xrp.tile([C + 1, SBF], BF16, tag="xs")
            nc.sync.dma_start(xs[:], x_wm.ap()[:, sb * SBF : (sb + 1) * SBF])
            for j in range(4):  # 4 chunks of 4 w-cols
                ps = psA.tile([M, 512], F32, tag="ps1")
                nc.tensor.matmul(
                    ps[:], w1t_sb[:], xs[:, j * 512 : (j + 1) * 512],
                    start=True, stop=True,
                )
                w0 = sb * SBW + 4 * j
                nc.scalar.activation(
                    _ap(R, (1 + w0) * RP + 1, [[RP * RP, M], [RP, 4], [1, H]]),
                    ps[:],
                    mybir.ActivationFunctionType.Relu,
                )

        def conv3x3_sb(sb):
            """conv3x3 + exp + transposes for superblock sb -> fr2 columns."""
            fs = frg.tile([SK, SBF], BF16, tag="fs")
            # tap-outer over the superblock's 4 chunks (4 PSUM banks)
            pss = [psB.tile([SK, 512], F32, tag=f"ps2_{j}") for j in range(4)]
            for t in range(9):
                ty, tx = divmod(t, 3)
                for j in range(4):
                    nc.tensor.matmul(
                        pss[j][:],
                        wet_sb[:, t * SK : (t + 1) * SK],
                        _ap(R, (sb * SBW + 4 * j + tx) * RP + ty,
                            [[RP * RP, M + 1], [RP, 4], [1, H]]),
                        start=(t == 0),
                        stop=(t == 8),
                    )
            for j in range(4):
                nc.scalar.activation(
                    fs[:, j * 512 : (j + 1) * 512],
                    pss[j][:],
                    mybir.ActivationFunctionType.Exp,
                )
            for wi in range(SBW):
                pst = psF.tile([128, SK], BF16, tag="pst")
                nc.tensor.transpose(
                    pst[:], fs[:, wi * H : (wi + 1) * H], ident[:]
                )
                nc.scalar.copy(
                    _ap(fr2, (sb * SBW + wi) * 2, [[F2, 128], [W * 2, SK], [1, 2]]),
                    _ap(pst, 0, [[SK, 128], [1, SK], [0, 2]]),
                )

        def z_half(half):
            """softmax denominator + reciprocal for w-columns of `half`."""
            for s in range(S2):
                nc.vector.reduce_sum(
                    zbuf[:, s * W + half * WH : s * W + (half + 1) * WH],
                    _ap(fr2, (s * K2 * W + half * WH) * 2,
                        [[F2, 128], [2, WH], [W * 2, K2]]),
                    axis=mybir.AxisListType.X,
                )
            nc.vector.reciprocal(
                _ap(zbuf, half * WH, [[S2 * W, 128], [W, S2], [1, WH]]),
                _ap(zbuf, half * WH, [[S2 * W, 128], [W, S2], [1, WH]]),
            )

        GROUPS5 = [list(range(5 * i, 5 * i + 5)) for i in range(5)]
        # 9 pairs + 7 singles: balances DVE in-group adds vs PE merge rate;
        # interleaved so the p-ring never outruns the PE merge of ring-buf n-2
        _pairs = [[2 * i, 2 * i + 1] for i in range(9)]
        _singles = [[k] for k in range(18, 25)]
        GROUPS16 = []
        for i in range(9):
            GROUPS16.append(_pairs[i])
            if i < 7:
                GROUPS16.append(_singles[i])

        def apply_units(half, xtd, pp_, tp_, abfp, psc, groups_of, a32p=None):
            """4 (s, half) apply units: DVE taps, partial-merges on PE
            (fp32 PSUM) or on DVE (fp32 SBUF) while PE is busy convolving,
            normalizing DVE epilogue, pixel-shuffle DMA out."""
            dims_u = [[FH, 128], [WH * 2, CH], [2, WH], [1, 2]]
            base_dx = 0 if half == 0 else 2
            for s in range(S2):
                groups, dve_merge = groups_of(s)
                acc = (
                    a32p.tile([128, FH], F32, tag="acc32")
                    if dve_merge
                    else psc.tile([128, FH], F32, tag="psacc")
                )
                for gi, grp in enumerate(groups):
                    p = pp_.tile([128, FH], BF16, tag="p")
                    last = len(grp) - 1
                    for n, k in enumerate(grp):
                        dyi, dxi = divmod(k, 5)
                        in0 = _ap(
                            xtd[dyi], (base_dx + dxi) * 2,
                            [[XFH, 128], [WHQ * 2, CH], [2, WH], [1, 2]],
                        )
                        in1 = _ap(
                            fr2, ((s * K2 + k) * W + half * WH) * 2,
                            [[F2, 128], [0, CH], [2, WH], [1, 2]],
                        )
                        if n == 0:
                            nc.vector.tensor_mul(_ap(p, 0, dims_u), in0, in1)
                        else:
                            t = tp_.tile([128, FH], BF16, tag="t")
                            nc.vector.tensor_mul(_ap(t, 0, dims_u), in0, in1)
                            if dve_merge and gi == 0 and n == last:
                                nc.vector.tensor_add(acc[:], p[:], t[:])
                            else:
                                nc.vector.tensor_add(p[:], p[:], t[:])
                    if dve_merge:
                        if gi > 0:
                            nc.vector.tensor_add(acc[:], acc[:], p[:])
                    else:
                        # accumulate the partial into fp32 PSUM on TensorE
                        for sl in range(8):
                            nc.tensor.matmul(
                                acc[:, sl * 512 : (sl + 1) * 512],
                                id128[:],
                                p[:, sl * 512 : (sl + 1) * 512],
                                start=(gi == 0),
                                stop=(gi == len(groups) - 1),
                                skip_group_check=True,
                            )
                # normalize + round once: accbf = acc * recipZ
                accbf = abfp.tile([128, FH], BF16, tag="accbf")
                nc.vector.tensor_mul(
                    _ap(accbf, 0, dims_u),
                    _ap(acc, 0, dims_u),
                    _ap(zbuf, s * W + half * WH,
                        [[S2 * W, 128], [0, CH], [1, WH], [0, 2]]),
                )
                # pixel-shuffle output: c = 2*c_hi + c_lo; c4 = c_hi//2,
                # c2 = c_hi%2, c1 = c_lo; rows 2h+c2, cols (2w+c1)
                for c2 in range(2):
                    dst = bass.AP(
                        tensor=out,
                        offset=(s * 16) * (4 * HW) + c2 * (2 * W) + half * (2 * WH),
                        ap=[[2 * (2 * W), 128], [4 * HW, 16], [1, 2 * WH]],
                    )
                    src = _ap(
                        accbf, c2 * (WH * 2),
                        [[FH, 128], [2 * (WH * 2), 16], [1, 2 * WH]],
                    )
                    nc.sync.dma_start(dst, src)

        # conv pipeline superblocks 0-3; Zh0 emitted as soon as its fr2
        # columns exist so its range dependencies stay tight
        conv1x1_sb(0)
        for sb in range(1, 5):
            conv1x1_sb(sb)
            conv3x3_sb(sb - 1)
        z_half(0)
        for sb in range(5, NSB):
            conv1x1_sb(sb)
            conv3x3_sb(sb - 1)
        conv3x3_sb(NSB - 1)
        psF.close()
        psB.close()
        psA.close()

        # half 0 apply overlaps the tail of the conv pipeline; the first
        # unit merges its 5-tap partials on the DVE (TensorE is still
        # convolving), later units offload merges to PE via small groups
        pp_ = _Pool(tc, name="pbuf", bufs=3)
        tp_ = _Pool(tc, name="tbuf", bufs=1)
        abfp = _Pool(tc, name="accbf", bufs=1)
        a32p = _Pool(tc, name="acc32", bufs=1)
        psc = _Pool(tc, name="psacc", bufs=1, space="PSUM")
        apply_units(0, xtd0, pp_, tp_, abfp, psc,
                    lambda s: (GROUPS5, True) if s == 0 else (GROUPS16, False),
                    a32p=a32p)
        psc.close()
        a32p.close()
        abfp.close()
        tp_.close()
        pp_.close()
        frg.close()
        xrp.close()
        rp_.close()
        xh0p.close()

        # half 1: X^T half tiles stream in while half 0 finishes
        xh1p = _Pool(tc, name="xth1", bufs=1)
        xtd1 = load_xtd(xh1p, 1)
        pp2 = _Pool(tc, name="pbuf2", bufs=3)
        tp2 = _Pool(tc, name="tbuf2", bufs=1)
        abf2 = _Pool(tc, name="accbf2", bufs=2)
        psc2 = _Pool(tc, name="psacc2", bufs=1, space="PSUM")
        z_half(1)
        apply_units(1, xtd1, pp2, tp2, abf2, psc2, lambda s: (GROUPS16, False))
        psc2.close()
        abf2.close()
        tp2.close()
        pp2.close()
        xh1p.close()

        if debug:
            nc.sync.dma_start(dbg["FR2"].ap(), fr2[:])
            nc.sync.dma_start(dbg["Z"].ap(), zbuf[:])
        zp.close()
        fr2p.close()
        cp.close()
    nc.compile()
    return nc, dbg


def host_inputs(x_img, w_compress, b_compress, w_encoder, b_encoder):
    """Per-core input map for one image [C, H, W] (all bf16)."""
    w1t = np.concatenate(
        [w_compress[:, :, 0, 0].T, b_compress[None, :]], axis=0
    ).astype(BF_NP)
    wet = np.zeros((M + 1, 9, SK), np.float32)
    for ty in range(3):
        for tx in range(3):
            wet[:M, ty * 3 + tx, :] = w_encoder[:, :, ty, tx].T
    wet[M, 4, :] = b_encoder
    x_wm = np.ones((C + 1, HW), np.float32)
    x_wm[:C] = x_img.transpose(0, 2, 1).reshape(C, HW)  # w-major
    # xtq[j, c_hi, wq, c_lo] = xpad[2*c_hi + c_lo, j-2, wq-2]
    xtq = np.zeros((WQ, CH, WQ, 2), np.float32)
    xtq[2:130, :, 2:130, :] = (
        x_img.reshape(CH, 2, H, W).transpose(2, 0, 3, 1)
    )
    return {
        "x_wm": x_wm.astype(BF_NP),
        "w1t": w1t,
        "wet": wet.reshape(M + 1, 9 * SK).astype(BF_NP),
        "xtq": xtq.reshape(WQ, XF).astype(BF_NP),
        "onesr": np.ones((1, RP * RP), BF_NP),
    }


_CACHE = {}


def kernel(x, w_compress, b_compress, w_encoder, b_encoder):
    x = np.asarray(x, np.float32)
    if "nc" not in _CACHE:
        _CACHE["nc"], _ = build_program(debug=False)
    nc = _CACHE["nc"]
    in_maps = [
        host_inputs(
            x[i],
            np.asarray(w_compress, np.float32),
            np.asarray(b_compress, np.float32),
            np.asarray(w_encoder, np.float32),
            np.asarray(b_encoder, np.float32),
        )
        for i in range(N_CORES)
    ]
    from concourse.bass_utils import run_bass_kernel_spmd

    res = run_bass_kernel_spmd(nc, in_maps, core_ids=list(range(N_CORES)))
    return np.stack(
        [res.results[i]["out"].astype(np.float32) for i in range(N_CORES)], axis=0
    )
